# revision 1
# baseline (speedup 1.0000x reference)
"""Multi-head attention (B=2, S=2048, E=1024, H=16, D=64) on 8 trn2 cores.

Sharding: core c = (b, g) with b = c // 4 (batch), g = c % 4 (head group of
4 heads = 256 features). Each core computes Q/K/V projections for its head
group, full attention for its 4 heads, and a partial output projection
(columns of its group); a ReduceScatter over the 4 cores of each batch sums
the partials and leaves each core with a [512, 1024] slice of the final
output. The host concatenates the slices and adds bo.

Device-side layouts (host pre-transposes/casts):
  xT  [1024, 2048]  query[b].T                 (compute dtype)
  wqT/wkT/wvT [1024, 256]  W[g*256:(g+1)*256, :].T
  woT [256, 1024]          Wo[:, g*256:(g+1)*256].T
  bq_r/bk_r/bv_r [1, 256]  bias rows (folded into matmuls as rank-1 updates)

On-chip dataflow per core (all contractions on the partition dim):
  Q^T,K^T [f,s] = (W^T chunk).T @ x^T      V [k,f] = (x^T chunk).T @ W^T
  S^T [k,q] = (K^T chunk).T @ Q^T   (K = d = 64)
  P^T = exp(S^T / 8)  via ScalarE, PSUM -> SBUF, cast to compute dtype
  O'^T [d+1,q] = (V' chunk).T @ P^T  with V' = [V | 1] (row d = softmax denom)
  O^T = O'^T[0:d] * broadcast(1/denom)   (broadcast via ones outer product)
  Y [s,f] = (O^T chunk).T @ Wo^T  -> ReduceScatter(+) over the 4-core group
"""

import numpy as np

B, S, E, H, D = 2, 2048, 1024, 16, 64
G = 4            # head groups (tensor-parallel)
GH = H // G      # heads per group = 4
GF = GH * D      # features per group = 256
NC = 8
SCALE = 1.0 / np.sqrt(D)

_CACHE = {}


def _build(mode: str, collective: bool, reps: int = 1):
    import concourse.bass as bass
    import concourse.mybir as mybir
    import concourse.tile as tile
    from concourse import bacc

    dt = mybir.dt
    C = {"bf16": dt.bfloat16, "f32r": dt.float32r, "fp32": dt.float32}[mode]
    f32 = dt.float32

    nc = bacc.Bacc()

    xT = nc.dram_tensor("xT", [E, S], C, kind="ExternalInput")
    wqT = nc.dram_tensor("wqT", [E, GF], C, kind="ExternalInput")
    wkT = nc.dram_tensor("wkT", [E, GF], C, kind="ExternalInput")
    wvT = nc.dram_tensor("wvT", [E, GF], C, kind="ExternalInput")
    woT = nc.dram_tensor("woT", [GF, E], C, kind="ExternalInput")
    bq_r = nc.dram_tensor("bq_r", [1, GF], C, kind="ExternalInput")
    bk_r = nc.dram_tensor("bk_r", [1, GF], C, kind="ExternalInput")
    bv_r = nc.dram_tensor("bv_r", [1, GF], C, kind="ExternalInput")
    ones512 = nc.dram_tensor("ones512", [1, 512], C, kind="ExternalInput")
    ones64 = nc.dram_tensor("ones64", [1, D], dt.float16, kind="ExternalInput")
    if collective:
        yout = nc.dram_tensor("yout", [S // G, E], f32, kind="ExternalOutput")
    else:
        yout = nc.dram_tensor("yout", [S, E], f32, kind="ExternalOutput")

    EC = E // 128    # 8 e-chunks
    SB = S // 128    # 16 s-blocks
    QC = S // 512    # 4 q-chunks
    KB = S // 128    # 16 k-blocks
    VW = GH * (D + 1)  # 260: V' row width (per head: 64 data + 1 ones col)
    KGS = [2] * 8  # k-block groups per exp call (sum = 16)

    def emit_body(nc, tc, res, do_coll):
        # per-e-chunk resident tiles for fine-grained deps
        xT_sb = [res.tile([128, S], C, tag=f"xT{ec}", name=f"xT{ec}")
                 for ec in range(EC)]
        wqT_sb = res.tile([128, EC * GF], C, tag="wqT")
        wkT_sb = res.tile([128, EC * GF], C, tag="wkT")
        wvT_sb = res.tile([128, EC * GF], C, tag="wvT")
        woT_sb = res.tile([128, 2 * E], C, tag="woT")
        # Q^T/K^T split per (fb, qc): tile [128, 512]
        QT_sb = [[res.tile([128, 512], C, tag=f"QT{fb}_{qc}", name=f"QT{fb}_{qc}")
                  for qc in range(QC)] for fb in range(2)]
        KT_sb = [[res.tile([128, 512], C, tag=f"KT{fb}_{qc}", name=f"KT{fb}_{qc}")
                  for qc in range(QC)] for fb in range(2)]
        V_sb = [res.tile([128, VW], C, tag=f"V{kb}", name=f"V{kb}")
                for kb in range(KB)]
        OT_sb = [[res.tile([128, 512], C, tag=f"OT{hb}_{qc}", name=f"OT{hb}_{qc}")
                  for qc in range(QC)] for hb in range(2)]
        bq_sb = res.tile([1, GF], C, tag="bq")
        bk_sb = res.tile([1, GF], C, tag="bk")
        bv_sb = res.tile([1, GF], C, tag="bv")
        on512_sb = res.tile([1, 512], C, tag="on512")
        on_sb = res.tile([1, D], dt.float16, tag="on")

        # input DMAs: small constants first, then interleave K-weights with x
        nc.sync.dma_start(out=bk_sb[:], in_=bk_r[:])
        nc.sync.dma_start(out=bq_sb[:], in_=bq_r[:])
        nc.sync.dma_start(out=bv_sb[:], in_=bv_r[:])
        nc.sync.dma_start(out=on512_sb[:], in_=ones512[:])
        nc.sync.dma_start(out=on_sb[:], in_=ones64[:])
        for ec in range(EC):
            nc.sync.dma_start(
                out=wkT_sb[:, ec * GF:(ec + 1) * GF],
                in_=wkT[ec * 128:(ec + 1) * 128, :])
            nc.sync.dma_start(out=xT_sb[ec][:],
                              in_=xT[ec * 128:(ec + 1) * 128, :])
        for w_sb, w_dr in ((wqT_sb, wqT), (wvT_sb, wvT)):
            for ec in range(EC):
                nc.sync.dma_start(
                    out=w_sb[:, ec * GF:(ec + 1) * GF],
                    in_=w_dr[ec * 128:(ec + 1) * 128, :])
        for ec in range(2):
            nc.sync.dma_start(
                out=woT_sb[:, ec * E:(ec + 1) * E],
                in_=woT[ec * 128:(ec + 1) * 128, :])

        # ones columns of V'
        for kb in range(KB):
            nc.gpsimd.memset(
                V_sb[kb][:].rearrange("p (h x) -> p h x", x=D + 1)[:, :, D:D + 1],
                1.0)

        # ---- projection / attention emit helpers ----
        def emit_qk_proj(pp, w_sb, dst, b_sb, fb, qc):
            # Q^T / K^T group in [f, s] layout: stationary = W^T chunk
            pq = pp.tile([128, 512], f32, tag="pot", name="pq", bufs=4)
            for ec in range(EC):
                nc.tensor.matmul(
                    pq[:],
                    lhsT=w_sb[:, ec * GF + fb * 128:ec * GF + fb * 128 + 128],
                    rhs=xT_sb[ec][:, qc * 512:qc * 512 + 512],
                    start=(ec == 0), stop=False)
            # bias as rank-1 update: bias-row.T @ ones-row
            nc.tensor.matmul(
                pq[:],
                lhsT=b_sb[:, fb * 128:(fb + 1) * 128],
                rhs=on512_sb[:],
                start=False, stop=True)
            nc.vector.tensor_copy(dst[fb][qc][:], pq[:])

        def emit_v_proj(pp, kb):
            # V group in natural [k, f] layout: stationary = x^T chunk
            pv = pp.tile([128, GF], f32, tag="pst", name="pv")
            for ec in range(EC):
                nc.tensor.matmul(
                    pv[:],
                    lhsT=xT_sb[ec][:, kb * 128:kb * 128 + 128],
                    rhs=wvT_sb[:, ec * GF:(ec + 1) * GF],
                    start=(ec == 0), stop=False)
            nc.tensor.matmul(
                pv[:],
                lhsT=on512_sb[:, 0:128],
                rhs=bv_sb[:],
                start=False, stop=True)
            nc.vector.tensor_copy(
                V_sb[kb][:].rearrange("p (h x) -> p h x", x=D + 1)[:, :, 0:D],
                pv.rearrange("p (h d) -> p h d", d=D))

        def emit_scores_exp(ps, ptp, h, qc):
            hb, hr = h // 2, (h % 2) * D
            ptt = ptp.tile([128, KB * 512], C, tag="ptt", name="ptt")
            kb0 = 0
            for kgs in KGS:
                pst = ps.tile([128, 2 * 512], f32, tag="pst", name="pst")
                for kj in range(kgs):
                    kb = kb0 + kj
                    nc.tensor.matmul(
                        pst[:, kj * 512:(kj + 1) * 512],
                        lhsT=KT_sb[hb][kb // 4][hr:hr + D,
                                                (kb % 4) * 128:
                                                (kb % 4) * 128 + 128],
                        rhs=QT_sb[hb][qc][hr:hr + D, :],
                        start=True, stop=True)
                nc.scalar.activation(
                    ptt[:, kb0 * 512:(kb0 + kgs) * 512],
                    pst[:, 0:kgs * 512],
                    mybir.ActivationFunctionType.Exp, scale=SCALE)
                kb0 += kgs
            return ptt

        def emit_av_norm(po, recp, h, qc, ptt):
            # O'^T accumulation; rows D.. of the same PSUM bank then hold
            # the broadcast reciprocal (outer product with ones)
            hb, hr = h // 2, (h % 2) * D
            pot = po.tile([128, 512], f32, tag="pot", name="pot", bufs=4)
            for kb in range(KB):
                nc.tensor.matmul(
                    pot[0:D + 1, :],
                    lhsT=V_sb[kb][:, h * (D + 1):(h + 1) * (D + 1)],
                    rhs=ptt[:, kb * 512:(kb + 1) * 512],
                    start=(kb == 0), stop=(kb == KB - 1))
            rec = recp.tile([1, 512], dt.float16, tag="rec", name="rec")
            with nc.allow_low_precision("fp16 softmax-denominator broadcast"):
                nc.vector.reciprocal(rec[:], pot[D:D + 1, :])
            nc.tensor.matmul(pot[D:D + D, :], lhsT=on_sb[:], rhs=rec[:],
                             start=True, stop=True)
            bc = recp.tile([D, 512], f32, tag="bc", name="bc")
            nc.vector.tensor_copy(bc[:], pot[D:D + D, :])
            nc.vector.tensor_tensor(
                out=OT_sb[hb][qc][hr:hr + D, :],
                in0=pot[0:D, :], in1=bc[:],
                op=mybir.AluOpType.mult)

        def emit_outproj_sb(po, ysb, sb):
            qc = sb // 4
            for fc in range(2):
                pyt = po.tile([128, 512], f32, tag="pot", name="pyt", bufs=4)
                for ec in range(2):
                    nc.tensor.matmul(
                        pyt[:],
                        lhsT=OT_sb[ec][qc][:, (sb % 4) * 128:
                                           (sb % 4) * 128 + 128],
                        rhs=woT_sb[:, ec * E + fc * 512:ec * E + fc * 512 + 512],
                        start=(ec == 0), stop=(ec == 1))
                yt = ysb.tile([128, 512], f32, tag="yt", name="yt")
                nc.vector.tensor_copy(yt[:], pyt[:])
                dst = y_part if collective else yout
                nc.sync.dma_start(
                    out=dst[sb * 128:(sb + 1) * 128, fc * 512:(fc + 1) * 512],
                    in_=yt[:])

        # ---- emission ----
        # One shared PSUM pool, 8 banks total by tag:
        #   "pot" [128,512] x2 bufs (pq/pot/pyt)   = 2 banks
        #   "pst" [128,1536] x2 bufs (pv/pst)      = 6 banks
        with tc.tile_pool(name="dram", bufs=1, space="DRAM") as dram, \
             tc.tile_pool(name="pall", bufs=2, space="PSUM") as pall, \
             tc.tile_pool(name="ptp", bufs=4) as ptp, \
             tc.tile_pool(name="recp", bufs=3) as recp, \
             tc.tile_pool(name="ysb", bufs=4) as ysb:
            if collective:
                y_part = dram.tile([S, E], f32, tag="ypart")
                rs_out = dram.tile([S // G, E], f32, tag="rsout")
            # Emission order = scheduler priority.  Interleave the first
            # q-chunk's scores/exp into the projections so ACT starts early;
            # delay each out-projection one q-chunk so it fills PE idle time
            # instead of starving ACT at chunk boundaries.
            ptts = {}
            for qc in range(QC):
                emit_qk_proj(pall, wkT_sb, KT_sb, bk_sb, 0, qc)
            emit_qk_proj(pall, wqT_sb, QT_sb, bq_sb, 0, 0)
            ptts[0] = emit_scores_exp(pall, ptp, 0, 0)
            ptts[1] = emit_scores_exp(pall, ptp, 1, 0)
            for qc in range(QC):
                emit_qk_proj(pall, wkT_sb, KT_sb, bk_sb, 1, qc)
            emit_qk_proj(pall, wqT_sb, QT_sb, bq_sb, 1, 0)
            ptts[2] = emit_scores_exp(pall, ptp, 2, 0)
            ptts[3] = emit_scores_exp(pall, ptp, 3, 0)
            emit_qk_proj(pall, wqT_sb, QT_sb, bq_sb, 0, 1)
            emit_qk_proj(pall, wqT_sb, QT_sb, bq_sb, 1, 1)
            for kb in range(KB):
                emit_v_proj(pall, kb)
            prev = [(h, 0, ptts[h]) for h in range(GH)]
            for qc in range(1, QC):
                pa = emit_scores_exp(pall, ptp, 0, qc)
                pb_ = emit_scores_exp(pall, ptp, 1, qc)
                for (ph, pqc, pt) in prev[:2]:
                    emit_av_norm(pall, recp, ph, pqc, pt)
                pc = emit_scores_exp(pall, ptp, 2, qc)
                pd = emit_scores_exp(pall, ptp, 3, qc)
                for (ph, pqc, pt) in prev[2:]:
                    emit_av_norm(pall, recp, ph, pqc, pt)
                prev = [(0, qc, pa), (1, qc, pb_), (2, qc, pc), (3, qc, pd)]
                if qc < QC - 1:
                    emit_qk_proj(pall, wqT_sb, QT_sb, bq_sb, 0, qc + 1)
                    emit_qk_proj(pall, wqT_sb, QT_sb, bq_sb, 1, qc + 1)
                for sb in range((qc - 1) * 4, (qc - 1) * 4 + 4):
                    emit_outproj_sb(pall, ysb, sb)
            for (ph, pqc, pt) in prev:
                emit_av_norm(pall, recp, ph, pqc, pt)
            for sb in range((QC - 1) * 4, (QC - 1) * 4 + 4):
                emit_outproj_sb(pall, ysb, sb)

            if collective and do_coll:
                nc.gpsimd.collective_compute(
                    "ReduceScatter",
                    mybir.AluOpType.add,
                    replica_groups=[[0, 1, 2, 3], [4, 5, 6, 7]],
                    ins=[y_part.opt()],
                    outs=[rs_out.opt()],
                )
                nc.sync.dma_start(out=yout[:], in_=rs_out[:])

    with tile.TileContext(nc) as tc:
        with tc.tile_pool(name="res", bufs=1) as res:
            for _rep in range(reps):
                emit_body(nc, tc, res, do_coll=(_rep == reps - 1))
    nc.finalize()
    return nc


def _np_dtype(mode):
    if mode == "bf16":
        import ml_dtypes
        return ml_dtypes.bfloat16
    return np.float32


def _in_maps(query, Wq, bq, Wk, bk, Wv, bv, Wo, bo, mode):
    ndt = _np_dtype(mode)
    maps = []
    for c in range(NC):
        b, g = c // G, c % G
        gr = slice(g * GF, (g + 1) * GF)
        maps.append({
            "xT": np.ascontiguousarray(query[b].T).astype(ndt),
            "wqT": np.ascontiguousarray(Wq[gr, :].T).astype(ndt),
            "wkT": np.ascontiguousarray(Wk[gr, :].T).astype(ndt),
            "wvT": np.ascontiguousarray(Wv[gr, :].T).astype(ndt),
            "woT": np.ascontiguousarray(Wo[:, gr].T).astype(ndt),
            "bq_r": np.asarray(bq[gr]).reshape(1, GF).astype(ndt),
            "bk_r": np.asarray(bk[gr]).reshape(1, GF).astype(ndt),
            "bv_r": np.asarray(bv[gr]).reshape(1, GF).astype(ndt),
            "ones512": np.ones((1, 512), ndt),
            "ones64": np.ones((1, D), np.float16),
        })
    return maps


def kernel(query, Wq, bq, Wk, bk, Wv, bv, Wo, bo,
           mode="bf16", collective=True, trace=False):
    from concourse.bass_utils import run_bass_kernel_spmd

    key = (mode, collective, 1)
    if key not in _CACHE:
        _CACHE[key] = _build(mode, collective)
    nc = _CACHE[key]

    maps = _in_maps(query, Wq, bq, Wk, bk, Wv, bv, Wo, bo, mode)
    res = run_bass_kernel_spmd(nc, maps, list(range(NC)), trace=trace)

    out = np.empty((B, S, E), np.float32)
    if collective:
        for c in range(NC):
            b, g = c // G, c % G
            out[b, g * (S // G):(g + 1) * (S // G), :] = res.results[c]["yout"]
    else:
        for b in range(B):
            out[b] = sum(res.results[b * G + g]["yout"] for g in range(G))
    out += np.asarray(bo, np.float32)
    if trace:
        kernel.last_results = res
    return out



# revision 2
# speedup vs baseline: 1.1244x; 1.1244x over previous
"""Multi-head attention (B=2, S=2048, E=1024, H=16, D=64) on 8 trn2 cores.

Sharding: core c = (b, g) with b = c // 4 (batch), g = c % 4 (head group of
4 heads = 256 features). Each core computes Q/K/V projections for its head
group, full attention for its 4 heads, and a partial output projection
(columns of its group); a ReduceScatter over the 4 cores of each batch sums
the partials and leaves each core with a [512, 1024] slice of the final
output. The host concatenates the slices and adds bo.

Device-side layouts (host pre-transposes/casts):
  xT  [1024, 2048]  query[b].T                 (compute dtype)
  wqT/wkT/wvT [1024, 256]  W[g*256:(g+1)*256, :].T
  woT [256, 1024]          Wo[:, g*256:(g+1)*256].T
  bq_c/bk_c [128, 2]       bias columns (fp32, added in the PSUM->SBUF copy)
  bv_r [1, 256]            bias row (folded into the V matmul as rank-1)
  ident [128, 128]         identity for PE transposes

On-chip dataflow per core (all contractions on the partition dim):
  Q^T,K^T [f,s] = (W^T chunk).T @ x^T + bias   (bias via DVE tensor_scalar)
  V [k,f] = (x^T chunk).T @ W^T (+ rank-1 bias)
  S^T [k,q] = (K^T chunk).T @ Q^T   (K = d = 64)
  P^T = exp(S^T / 8)  via ScalarE, PSUM -> SBUF, cast to compute dtype
  O  [q,d+1] = (P^T chunk).T @ V'   with V' = [V | 1] (col d = denom)
    -- flipped AV: stationary = P^T chunk, so the matmul's free dim is
       d+1 = 65 instead of 512, quartering tensor-engine time there.
  O <- O * (1/denom)  (DVE per-partition scalar multiply), then
  O^T via PE transpose (identity), staged back to SBUF
  Y [s,f] = (O^T chunk).T @ Wo^T  -> ReduceScatter(+) over the 4-core group
"""

import numpy as np

B, S, E, H, D = 2, 2048, 1024, 16, 64
G = 4            # head groups (tensor-parallel)
GH = H // G      # heads per group = 4
GF = GH * D      # features per group = 256
NC = 8
SCALE = 1.0 / np.sqrt(D)

_CACHE = {}


def _build(mode: str, collective: bool, reps: int = 1):
    import concourse.bass as bass
    import concourse.mybir as mybir
    import concourse.tile as tile
    from concourse import bacc

    dt = mybir.dt
    C = {"bf16": dt.bfloat16, "f32r": dt.float32r, "fp32": dt.float32}[mode]
    f32 = dt.float32

    nc = bacc.Bacc()

    xT = nc.dram_tensor("xT", [E, S], C, kind="ExternalInput")
    wqT = nc.dram_tensor("wqT", [E, GF], C, kind="ExternalInput")
    wkT = nc.dram_tensor("wkT", [E, GF], C, kind="ExternalInput")
    wvT = nc.dram_tensor("wvT", [E, GF], C, kind="ExternalInput")
    woT = nc.dram_tensor("woT", [GF, E], C, kind="ExternalInput")
    bq_c = nc.dram_tensor("bq_c", [128, 2], f32, kind="ExternalInput")
    bk_c = nc.dram_tensor("bk_c", [128, 2], f32, kind="ExternalInput")
    bv_r = nc.dram_tensor("bv_r", [1, GF], C, kind="ExternalInput")
    ones512 = nc.dram_tensor("ones512", [1, 512], C, kind="ExternalInput")
    ident = nc.dram_tensor("ident", [128, 128], C, kind="ExternalInput")
    if collective:
        yout = nc.dram_tensor("yout", [S // G, E], f32, kind="ExternalOutput")
    else:
        yout = nc.dram_tensor("yout", [S, E], f32, kind="ExternalOutput")

    EC = E // 128    # 8 e-chunks
    QC = S // 512    # 4 q-chunks
    KB = S // 128    # 16 k-blocks
    VW = GH * (D + 1)  # 260: V' row width (per head: 64 data + 1 ones col)
    KGS = [2] * 8  # k-block groups per exp call (sum = 16)

    def emit_body(nc, tc, res, do_coll):
        # per-e-chunk resident tiles for fine-grained deps
        xT_sb = [res.tile([128, S], C, tag=f"xT{ec}", name=f"xT{ec}")
                 for ec in range(EC)]
        wqT_sb = res.tile([128, EC * GF], C, tag="wqT")
        wkT_sb = res.tile([128, EC * GF], C, tag="wkT")
        wvT_sb = res.tile([128, EC * GF], C, tag="wvT")
        woT_sb = res.tile([128, 2 * E], C, tag="woT")
        # Q^T/K^T split per (fb, qc): tile [128, 512]
        QT_sb = [[res.tile([128, 512], C, tag=f"QT{fb}_{qc}", name=f"QT{fb}_{qc}")
                  for qc in range(QC)] for fb in range(2)]
        KT_sb = [[res.tile([128, 512], C, tag=f"KT{fb}_{qc}", name=f"KT{fb}_{qc}")
                  for qc in range(QC)] for fb in range(2)]
        V_sb = [res.tile([128, VW], C, tag=f"V{kb}", name=f"V{kb}")
                for kb in range(KB)]
        # O^T per qc: [128, 2*512]: free = hb*512 + q  (hb = head-pair block)
        OT2_sb = [res.tile([128, 2 * 512], C, tag=f"OT{qc}", name=f"OT{qc}")
                  for qc in range(QC)]
        bq_sb = res.tile([128, 2], f32, tag="bq")
        bk_sb = res.tile([128, 2], f32, tag="bk")
        bv_sb = res.tile([1, GF], C, tag="bv")
        on512_sb = res.tile([1, 512], C, tag="on512")
        id_sb = res.tile([128, 128], C, tag="ident")

        # input DMAs: small constants first, then interleave K-weights with x
        nc.sync.dma_start(out=bk_sb[:], in_=bk_c[:])
        nc.sync.dma_start(out=bq_sb[:], in_=bq_c[:])
        nc.sync.dma_start(out=bv_sb[:], in_=bv_r[:])
        nc.sync.dma_start(out=on512_sb[:], in_=ones512[:])
        nc.sync.dma_start(out=id_sb[:], in_=ident[:])
        for ec in range(EC):
            nc.sync.dma_start(
                out=wkT_sb[:, ec * GF:(ec + 1) * GF],
                in_=wkT[ec * 128:(ec + 1) * 128, :])
            nc.sync.dma_start(out=xT_sb[ec][:],
                              in_=xT[ec * 128:(ec + 1) * 128, :])
        for w_sb, w_dr in ((wqT_sb, wqT), (wvT_sb, wvT)):
            for ec in range(EC):
                nc.sync.dma_start(
                    out=w_sb[:, ec * GF:(ec + 1) * GF],
                    in_=w_dr[ec * 128:(ec + 1) * 128, :])
        for ec in range(2):
            nc.sync.dma_start(
                out=woT_sb[:, ec * E:(ec + 1) * E],
                in_=woT[ec * 128:(ec + 1) * 128, :])

        # ones columns of V'
        for kb in range(KB):
            nc.gpsimd.memset(
                V_sb[kb][:].rearrange("p (h x) -> p h x", x=D + 1)[:, :, D:D + 1],
                1.0)

        # ---- projection / attention emit helpers ----
        def emit_qk_proj(pp, w_sb, dst, b_sb, fb, qc):
            # Q^T / K^T group in [f, s] layout: stationary = W^T chunk
            pq = pp.tile([128, 512], f32, tag="pq", name="pq", bufs=2)
            for ec in range(EC):
                nc.tensor.matmul(
                    pq[:],
                    lhsT=w_sb[:, ec * GF + fb * 128:ec * GF + fb * 128 + 128],
                    rhs=xT_sb[ec][:, qc * 512:qc * 512 + 512],
                    start=(ec == 0), stop=(ec == EC - 1))
            # bias folded into the PSUM->SBUF copy as a per-partition scalar
            nc.vector.tensor_scalar_add(
                out=dst[fb][qc][:], in0=pq[:], scalar1=b_sb[:, fb:fb + 1])

        def emit_v_proj_pair(pp, j):
            # V group in natural [k, f] layout for k-blocks 2j, 2j+1
            pv = pp.tile([128, 512], f32, tag="pav", name="pv", bufs=2)
            for t in range(2):
                kb = 2 * j + t
                for ec in range(EC):
                    nc.tensor.matmul(
                        pv[:, t * GF:(t + 1) * GF],
                        lhsT=xT_sb[ec][:, kb * 128:kb * 128 + 128],
                        rhs=wvT_sb[:, ec * GF:(ec + 1) * GF],
                        start=(ec == 0), stop=False)
                nc.tensor.matmul(
                    pv[:, t * GF:(t + 1) * GF],
                    lhsT=on512_sb[:, 0:128],
                    rhs=bv_sb[:],
                    start=False, stop=True)
            for t in range(2):
                kb = 2 * j + t
                nc.vector.tensor_copy(
                    V_sb[kb][:].rearrange("p (h x) -> p h x", x=D + 1)[:, :, 0:D],
                    pv[:, t * GF:(t + 1) * GF].rearrange(
                        "p (h d) -> p h d", d=D))

        def emit_scores_exp(ps, ptp, h, qc):
            hb, hr = h // 2, (h % 2) * D
            ptt = ptp.tile([128, KB * 512], C, tag="ptt", name="ptt")
            kb0 = 0
            for kgs in KGS:
                pst = ps.tile([128, 2 * 512], f32, tag="pst", name="pst", bufs=2)
                for kj in range(kgs):
                    kb = kb0 + kj
                    nc.tensor.matmul(
                        pst[:, kj * 512:(kj + 1) * 512],
                        lhsT=KT_sb[hb][kb // 4][hr:hr + D,
                                                (kb % 4) * 128:
                                                (kb % 4) * 128 + 128],
                        rhs=QT_sb[hb][qc][hr:hr + D, :],
                        start=True, stop=True)
                nc.scalar.activation(
                    ptt[:, kb0 * 512:(kb0 + kgs) * 512],
                    pst[:, 0:kgs * 512],
                    mybir.ActivationFunctionType.Exp, scale=SCALE)
                kb0 += kgs
            return ptt

        def emit_av(pp, recp, O2, h, qc, ptt):
            # Flipped AV: stationary = P^T chunk [128k, 128q], moving = V'
            # [128k, 65]; out[q, 65] accumulates over the 16 k-blocks, with
            # col 64 = softmax denominator. One PSUM bank holds all 4
            # q-subblocks of this head (4 x 65 = 260 fp32 cols).
            pav = pp.tile([128, 512], f32, tag="pav", name="pav", bufs=2)
            for qs in range(4):
                for kb in range(KB):
                    nc.tensor.matmul(
                        pav[:, qs * (D + 1):(qs + 1) * (D + 1)],
                        lhsT=ptt[:, kb * 512 + qs * 128:kb * 512 + qs * 128 + 128],
                        rhs=V_sb[kb][:, h * (D + 1):(h + 1) * (D + 1)],
                        start=(kb == 0), stop=(kb == KB - 1))
            for qs in range(4):
                rec = recp.tile([128, 1], f32, tag="rec", name="rec")
                nc.vector.reciprocal(
                    rec[:], pav[:, qs * (D + 1) + D:qs * (D + 1) + D + 1])
                nc.vector.tensor_scalar_mul(
                    out=O2[qs][:, h * D:(h + 1) * D],
                    in0=pav[:, qs * (D + 1):qs * (D + 1) + D],
                    scalar1=rec[:])

        def emit_transposes(pp, O2, qc, hb, pT):
            # O [q, f-pair] -> O^T [f-pair, q] for head pair hb, all 4 qs.
            # pT free layout hb*512 + qs*128 matches OT2's ec*512 + q layout.
            for qs in range(4):
                nc.tensor.transpose(
                    out=pT[:, hb * 512 + qs * 128:hb * 512 + qs * 128 + 128],
                    in_=O2[qs][:, hb * 128:hb * 128 + 128],
                    identity=id_sb[:])

        def emit_outproj_sb(po, ysb, sb):
            qc = sb // 4
            for fc in range(2):
                pyt = po.tile([128, 512], f32, tag=("pav" if fc == 0 else "pq"),
                              name="pyt")
                for ec in range(2):
                    nc.tensor.matmul(
                        pyt[:],
                        lhsT=OT2_sb[qc][:, ec * 512 + (sb % 4) * 128:
                                        ec * 512 + (sb % 4) * 128 + 128],
                        rhs=woT_sb[:, ec * E + fc * 512:ec * E + fc * 512 + 512],
                        start=(ec == 0), stop=(ec == 1))
                yt = ysb.tile([128, 512], f32, tag="yt", name="yt")
                nc.vector.tensor_copy(yt[:], pyt[:])
                dst = y_part if collective else yout
                nc.sync.dma_start(
                    out=dst[sb * 128:(sb + 1) * 128, fc * 512:(fc + 1) * 512],
                    in_=yt[:])

        # ---- emission ----
        # PSUM (8 banks): "pst" [128,1024] x2 bufs = 4 banks (scores+exp),
        # "pq" [128,512] x2 bufs = 2 banks (projections, transposes, outproj),
        # "pav" [128,512] x2 bufs = 2 banks (V-proj, AV accum, outproj).
        with tc.tile_pool(name="dram", bufs=1, space="DRAM") as dram, \
             tc.tile_pool(name="pall", bufs=2, space="PSUM") as pall, \
             tc.tile_pool(name="ptp", bufs=5) as ptp, \
             tc.tile_pool(name="o2p", bufs=4) as o2p, \
             tc.tile_pool(name="recp", bufs=4) as recp, \
             tc.tile_pool(name="ysb", bufs=4) as ysb:
            if collective:
                y_part = dram.tile([S, E], f32, tag="ypart")
                rs_out = dram.tile([S // G, E], f32, tag="rsout")
            # Emission order = scheduler priority.  scores run just-in-time
            # ahead of each exp (pst ring), AV for qc-1 is interleaved between
            # the exps of qc so the ptt ring (bufs=5) always frees forward.
            ptts = {}
            for qc in range(QC):
                emit_qk_proj(pall, wkT_sb, KT_sb, bk_sb, 0, qc)
            emit_qk_proj(pall, wqT_sb, QT_sb, bq_sb, 0, 0)
            ptts[0] = emit_scores_exp(pall, ptp, 0, 0)
            ptts[1] = emit_scores_exp(pall, ptp, 1, 0)
            for qc in range(QC):
                emit_qk_proj(pall, wkT_sb, KT_sb, bk_sb, 1, qc)
            emit_qk_proj(pall, wqT_sb, QT_sb, bq_sb, 1, 0)
            ptts[2] = emit_scores_exp(pall, ptp, 2, 0)
            ptts[3] = emit_scores_exp(pall, ptp, 3, 0)
            for j in range(KB // 2):
                emit_v_proj_pair(pall, j)
            emit_qk_proj(pall, wqT_sb, QT_sb, bq_sb, 0, 1)
            emit_qk_proj(pall, wqT_sb, QT_sb, bq_sb, 1, 1)

            def emit_av_block(pqc, O2, pT):
                # AV + transposes + OT2 copy + outproj for q-chunk pqc,
                # interleaved with the exps of the next q-chunk by the caller
                nc.vector.tensor_copy(OT2_sb[pqc][:], pT[:])
                for sb in range(pqc * 4, pqc * 4 + 4):
                    emit_outproj_sb(pall, ysb, sb)

            for qc in range(1, QC):
                pqc = qc - 1
                O2 = [o2p.tile([128, GH * D], C, tag=f"o2_{qs}", name="o2")
                      for qs in range(4)]
                pT = pall.tile([128, 1024], C, tag="pq", name="pT")
                new_ptts = {}
                for h in range(GH):
                    new_ptts[h] = emit_scores_exp(pall, ptp, h, qc)
                    emit_av(pall, recp, O2, h, pqc, ptts[h])
                    if h == 1:
                        emit_transposes(pall, O2, pqc, 0, pT)
                emit_transposes(pall, O2, pqc, 1, pT)
                emit_av_block(pqc, O2, pT)
                ptts = new_ptts
                if qc < QC - 1:
                    emit_qk_proj(pall, wqT_sb, QT_sb, bq_sb, 0, qc + 1)
                    emit_qk_proj(pall, wqT_sb, QT_sb, bq_sb, 1, qc + 1)

            pqc = QC - 1
            O2 = [o2p.tile([128, GH * D], C, tag=f"o2_{qs}", name="o2")
                  for qs in range(4)]
            pT = pall.tile([128, 1024], C, tag="pq", name="pT")
            for h in range(GH):
                emit_av(pall, recp, O2, h, pqc, ptts[h])
                if h == 1:
                    emit_transposes(pall, O2, pqc, 0, pT)
            emit_transposes(pall, O2, pqc, 1, pT)
            emit_av_block(pqc, O2, pT)

            if collective and do_coll:
                nc.gpsimd.collective_compute(
                    "ReduceScatter",
                    mybir.AluOpType.add,
                    replica_groups=[[0, 1, 2, 3], [4, 5, 6, 7]],
                    ins=[y_part.opt()],
                    outs=[rs_out.opt()],
                )
                nc.sync.dma_start(out=yout[:], in_=rs_out[:])

    with tile.TileContext(nc) as tc:
        with tc.tile_pool(name="res", bufs=1) as res:
            for _rep in range(reps):
                emit_body(nc, tc, res, do_coll=(_rep == reps - 1))
    nc.finalize()
    return nc


def _np_dtype(mode):
    if mode == "bf16":
        import ml_dtypes
        return ml_dtypes.bfloat16
    return np.float32


def _in_maps(query, Wq, bq, Wk, bk, Wv, bv, Wo, bo, mode):
    ndt = _np_dtype(mode)
    maps = []
    for c in range(NC):
        b, g = c // G, c % G
        gr = slice(g * GF, (g + 1) * GF)
        maps.append({
            "xT": np.ascontiguousarray(query[b].T).astype(ndt),
            "wqT": np.ascontiguousarray(Wq[gr, :].T).astype(ndt),
            "wkT": np.ascontiguousarray(Wk[gr, :].T).astype(ndt),
            "wvT": np.ascontiguousarray(Wv[gr, :].T).astype(ndt),
            "woT": np.ascontiguousarray(Wo[:, gr].T).astype(ndt),
            "bq_c": np.ascontiguousarray(
                np.asarray(bq[gr], np.float32).reshape(2, 128).T),
            "bk_c": np.ascontiguousarray(
                np.asarray(bk[gr], np.float32).reshape(2, 128).T),
            "bv_r": np.asarray(bv[gr]).reshape(1, GF).astype(ndt),
            "ones512": np.ones((1, 512), ndt),
            "ident": np.eye(128, dtype=np.float32).astype(ndt),
        })
    return maps


def kernel(query, Wq, bq, Wk, bk, Wv, bv, Wo, bo,
           mode="bf16", collective=True, trace=False):
    from concourse.bass_utils import run_bass_kernel_spmd

    key = (mode, collective, 1)
    if key not in _CACHE:
        _CACHE[key] = _build(mode, collective)
    nc = _CACHE[key]

    maps = _in_maps(query, Wq, bq, Wk, bk, Wv, bv, Wo, bo, mode)
    res = run_bass_kernel_spmd(nc, maps, list(range(NC)), trace=trace)

    out = np.empty((B, S, E), np.float32)
    if collective:
        for c in range(NC):
            b, g = c // G, c % G
            out[b, g * (S // G):(g + 1) * (S // G), :] = res.results[c]["yout"]
    else:
        for b in range(B):
            out[b] = sum(res.results[b * G + g]["yout"] for g in range(G))
    out += np.asarray(bo, np.float32)
    if trace:
        kernel.last_results = res
    return out


# revision 4
# speedup vs baseline: 1.2074x; 1.0738x over previous
"""Multi-head attention (B=2, S=2048, E=1024, H=16, D=64) on 8 trn2 cores.

Sharding: core c = (b, g) with b = c // 4 (batch), g = c % 4 (head group of
4 heads = 256 features). Each core computes Q/K/V projections for its head
group, full attention for its 4 heads, and a partial output projection
(columns of its group); a ReduceScatter over the 4 cores of each batch sums
the partials and leaves each core with a [512, 1024] slice of the final
output. The host concatenates the slices and adds bo.

Device-side layouts (host pre-transposes/casts):
  xT  [1024, 2048]  query[b].T                 (compute dtype)
  wqT/wkT/wvT [1024, 256]  W[g*256:(g+1)*256, :].T
  woT [256, 1024]          Wo[:, g*256:(g+1)*256].T
  bq_c/bk_c [128, 2]       bias columns (fp32, added in the PSUM->SBUF copy)
  bv_r [1, 256]            bias row (folded into the V matmul as rank-1)
  ident [128, 128]         identity for PE transposes

On-chip dataflow per core (all contractions on the partition dim):
  Q^T,K^T [f,s] = (W^T chunk).T @ x^T + bias   (bias via DVE tensor_scalar)
  V [k,f] = (x^T chunk).T @ W^T (+ rank-1 bias)
  S^T [k,q] = (K^T chunk).T @ Q^T   (K = d = 64)
  P^T = exp(S^T / 8)  via ScalarE, PSUM -> SBUF, cast to compute dtype
  O  [q,d+1] = (P^T chunk).T @ V'   with V' = [V | 1] (col d = denom)
    -- flipped AV: stationary = P^T chunk, so the matmul's free dim is
       d+1 = 65 instead of 512, quartering tensor-engine time there.
  O <- O * (1/denom)  (DVE per-partition scalar multiply), then
  O^T via PE transpose (identity), staged back to SBUF
  Y [s,f] = (O^T chunk).T @ Wo^T  -> ReduceScatter(+) over the 4-core group

x is DMAed in four column groups (the first K/Q projection tiles only need
the first quarter of the sequence) and the emission order interleaves the
first head's score groups with the K projection so ScalarE starts ~10us in.
"""

import numpy as np

B, S, E, H, D = 2, 2048, 1024, 16, 64
G = 4            # head groups (tensor-parallel)
GH = H // G      # heads per group = 4
GF = GH * D      # features per group = 256
NC = 8
SCALE = 1.0 / np.sqrt(D)

_CACHE = {}


def _build(mode: str, collective: bool, reps: int = 1):
    import concourse.bass as bass
    import concourse.mybir as mybir
    import concourse.tile as tile
    from concourse import bacc

    dt = mybir.dt
    C = {"bf16": dt.bfloat16, "f32r": dt.float32r, "fp32": dt.float32}[mode]
    f32 = dt.float32

    nc = bacc.Bacc()

    xT = nc.dram_tensor("xT", [E, S], C, kind="ExternalInput")
    wqT = nc.dram_tensor("wqT", [E, GF], C, kind="ExternalInput")
    wkT = nc.dram_tensor("wkT", [E, GF], C, kind="ExternalInput")
    wvT = nc.dram_tensor("wvT", [E, GF], C, kind="ExternalInput")
    woT = nc.dram_tensor("woT", [GF, E], C, kind="ExternalInput")
    bq_c = nc.dram_tensor("bq_c", [128, 2], f32, kind="ExternalInput")
    bk_c = nc.dram_tensor("bk_c", [128, 2], f32, kind="ExternalInput")
    bv_r = nc.dram_tensor("bv_r", [1, GF], C, kind="ExternalInput")
    ones512 = nc.dram_tensor("ones512", [1, 512], C, kind="ExternalInput")
    ident = nc.dram_tensor("ident", [128, 128], C, kind="ExternalInput")
    if collective:
        yout = nc.dram_tensor("yout", [S // G, E], f32, kind="ExternalOutput")
    else:
        yout = nc.dram_tensor("yout", [S, E], f32, kind="ExternalOutput")

    EC = E // 128    # 8 e-chunks
    QC = S // 512    # 4 q-chunks
    KB = S // 128    # 16 k-blocks
    VW = GH * (D + 1)  # 260: V' row width (per head: 64 data + 1 ones col)

    def emit_body(nc, tc, res, do_coll):
        # x as one resident tile, e-chunk major; column-group DMAs fill it
        xAll = res.tile([128, EC * S], C, tag="xAll", name="xAll")

        def xs(ec, c0, c1):
            return xAll[:, ec * S + c0:ec * S + c1]

        wqT_sb = res.tile([128, EC * GF], C, tag="wqT")
        wkT_sb = res.tile([128, EC * GF], C, tag="wkT")
        wvT_sb = res.tile([128, EC * GF], C, tag="wvT")
        woT_sb = res.tile([128, 2 * E], C, tag="woT")
        QT_sb = [[res.tile([128, 512], C, tag=f"QT{fb}_{qc}", name=f"QT{fb}_{qc}")
                  for qc in range(QC)] for fb in range(2)]
        KT_sb = [[res.tile([128, 512], C, tag=f"KT{fb}_{qc}", name=f"KT{fb}_{qc}")
                  for qc in range(QC)] for fb in range(2)]
        V_sb = [res.tile([128, VW], C, tag=f"V{kb}", name=f"V{kb}")
                for kb in range(KB)]
        # O^T per qc: [128, 2*512]: free = hb*512 + q  (hb = head-pair block)
        OT2_sb = [res.tile([128, 2 * 512], C, tag=f"OT{qc}", name=f"OT{qc}")
                  for qc in range(QC)]
        bq_sb = res.tile([128, 2], f32, tag="bq")
        bk_sb = res.tile([128, 2], f32, tag="bk")
        bv_sb = res.tile([1, GF], C, tag="bv")
        on512_sb = res.tile([1, 512], C, tag="on512")
        id_sb = res.tile([128, 128], C, tag="ident")

        # input DMAs, ordered for the critical path: wk, x cols 0:512, wq,
        # qk biases, remaining x columns, then v/o weights and constants.
        def dma_w(dst, src, nech):
            nc.sync.dma_start(
                out=dst[:].rearrange("p (g f) -> p g f", g=nech),
                in_=src[:].rearrange("(g p) f -> p g f", p=128))

        def dma_xcols(qp):
            nc.sync.dma_start(
                out=xAll[:].rearrange("p (g s) -> p g s", g=EC)
                    [:, :, qp * 512:(qp + 1) * 512],
                in_=xT[:].rearrange("(g p) s -> p g s", p=128)
                    [:, :, qp * 512:(qp + 1) * 512])

        dma_w(wkT_sb, wkT, EC)
        dma_xcols(0)
        dma_w(wqT_sb, wqT, EC)
        nc.sync.dma_start(out=bk_sb[:], in_=bk_c[:])
        nc.sync.dma_start(out=bq_sb[:], in_=bq_c[:])
        for qp in range(1, 4):
            dma_xcols(qp)
        nc.sync.dma_start(out=bv_sb[:], in_=bv_r[:])
        nc.sync.dma_start(out=on512_sb[:], in_=ones512[:])
        nc.sync.dma_start(out=id_sb[:], in_=ident[:])
        dma_w(wvT_sb, wvT, EC)
        dma_w(woT_sb, woT, 2)

        # ones columns of V'
        for kb in range(KB):
            nc.gpsimd.memset(
                V_sb[kb][:].rearrange("p (h x) -> p h x", x=D + 1)[:, :, D:D + 1],
                1.0)

        # ---- emit helpers ----
        def emit_qk_proj(pp, w_sb, dst, b_sb, fb, qc):
            # Q^T / K^T group in [f, s] layout: stationary = W^T chunk
            pq = pp.tile([128, 512], f32, tag="pq", name="pq", bufs=2)
            for ec in range(EC):
                nc.tensor.matmul(
                    pq[:],
                    lhsT=w_sb[:, ec * GF + fb * 128:ec * GF + fb * 128 + 128],
                    rhs=xs(ec, qc * 512, qc * 512 + 512),
                    start=(ec == 0), stop=(ec == EC - 1))
            # bias folded into the PSUM->SBUF copy as a per-partition scalar
            nc.vector.tensor_scalar_add(
                out=dst[fb][qc][:], in0=pq[:], scalar1=b_sb[:, fb:fb + 1])

        def emit_v_proj_pair(pp, j):
            # V group in natural [k, f] layout for k-blocks 2j, 2j+1
            pv = pp.tile([128, 512], f32, tag="pav", name="pv", bufs=2)
            for t in range(2):
                kb = 2 * j + t
                for ec in range(EC):
                    nc.tensor.matmul(
                        pv[:, t * GF:(t + 1) * GF],
                        lhsT=xs(ec, kb * 128, kb * 128 + 128),
                        rhs=wvT_sb[:, ec * GF:(ec + 1) * GF],
                        start=(ec == 0), stop=False)
                nc.tensor.matmul(
                    pv[:, t * GF:(t + 1) * GF],
                    lhsT=on512_sb[:, 0:128],
                    rhs=bv_sb[:],
                    start=False, stop=True)
            for t in range(2):
                kb = 2 * j + t
                nc.vector.tensor_copy(
                    V_sb[kb][:].rearrange("p (h x) -> p h x", x=D + 1)[:, :, 0:D],
                    pv[:, t * GF:(t + 1) * GF].rearrange(
                        "p (h d) -> p h d", d=D))

        def emit_scores_exp(ps, h, qc, ptt, groups):
            hb, hr = h // 2, (h % 2) * D
            for gi in groups:
                kb0 = 2 * gi
                pst = ps.tile([128, 2 * 512], f32, tag="pst", name="pst", bufs=2)
                for kj in range(2):
                    kb = kb0 + kj
                    nc.tensor.matmul(
                        pst[:, kj * 512:(kj + 1) * 512],
                        lhsT=KT_sb[hb][kb // 4][hr:hr + D,
                                                (kb % 4) * 128:
                                                (kb % 4) * 128 + 128],
                        rhs=QT_sb[hb][qc][hr:hr + D, :],
                        start=True, stop=True)
                nc.scalar.activation(
                    ptt[:, kb0 * 512:(kb0 + 2) * 512],
                    pst[:],
                    mybir.ActivationFunctionType.Exp, scale=SCALE)

        def emit_av(pp, recp, O2, h, ptt):
            # Flipped AV: stationary = P^T chunk [128k, 128q], moving = V'
            # [128k, 65]; out[q, 65] accumulates over the 16 k-blocks, with
            # col 64 = softmax denominator. One PSUM bank holds all 4
            # q-subblocks of this head (4 x 65 = 260 fp32 cols).
            pav = pp.tile([128, 512], f32, tag="pav", name="pav", bufs=2)
            for qs in range(4):
                for kb in range(KB):
                    nc.tensor.matmul(
                        pav[:, qs * (D + 1):(qs + 1) * (D + 1)],
                        lhsT=ptt[:, kb * 512 + qs * 128:kb * 512 + qs * 128 + 128],
                        rhs=V_sb[kb][:, h * (D + 1):(h + 1) * (D + 1)],
                        start=(kb == 0), stop=(kb == KB - 1))
            for qs in range(4):
                rec = recp.tile([128, 1], f32, tag="rec", name="rec")
                nc.vector.reciprocal(
                    rec[:], pav[:, qs * (D + 1) + D:qs * (D + 1) + D + 1])
                nc.vector.tensor_scalar_mul(
                    out=O2[qs][:, h * D:(h + 1) * D],
                    in0=pav[:, qs * (D + 1):qs * (D + 1) + D],
                    scalar1=rec[:])

        def emit_transposes(pp, O2, qc, hb):
            # O [q, f-pair] -> O^T [f-pair, q] for head pair hb, all 4 qs,
            # into a short-lived psum tile, then one DVE copy to OT2.
            pT = pp.tile([128, 512], C, tag="pq", name="pT")
            for qs in range(4):
                nc.tensor.transpose(
                    out=pT[:, qs * 128:qs * 128 + 128],
                    in_=O2[qs][:, hb * 128:hb * 128 + 128],
                    identity=id_sb[:])
            nc.vector.tensor_copy(
                OT2_sb[qc][:, hb * 512:(hb + 1) * 512], pT[:])

        def emit_outproj_sb(po, ysb, sb):
            qc = sb // 4
            for fc in range(2):
                pyt = po.tile([128, 512], f32, tag=("pav" if fc == 0 else "pq"),
                              name="pyt")
                for ec in range(2):
                    nc.tensor.matmul(
                        pyt[:],
                        lhsT=OT2_sb[qc][:, ec * 512 + (sb % 4) * 128:
                                        ec * 512 + (sb % 4) * 128 + 128],
                        rhs=woT_sb[:, ec * E + fc * 512:ec * E + fc * 512 + 512],
                        start=(ec == 0), stop=(ec == 1))
                yt = ysb.tile([128, 512], f32, tag="yt", name="yt")
                nc.vector.tensor_copy(yt[:], pyt[:])
                dst = y_part if collective else yout
                nc.sync.dma_start(
                    out=dst[sb * 128:(sb + 1) * 128, fc * 512:(fc + 1) * 512],
                    in_=yt[:])

        # ---- emission (order = scheduler priority; engine queues are
        # in-order, so every dependency must appear before its consumer) ----
        # PSUM (8 banks): "pst" [128,1024] x2 = 4 banks (scores+exp),
        # "pq" [128,512] x2 = 2 banks (projections, pT staging, outproj),
        # "pav" [128,512] x2 = 2 banks (V-proj, AV accum, outproj).
        with tc.tile_pool(name="dram", bufs=1, space="DRAM") as dram, \
             tc.tile_pool(name="pall", bufs=2, space="PSUM") as pall, \
             tc.tile_pool(name="ptp", bufs=5) as ptp, \
             tc.tile_pool(name="o2p", bufs=4) as o2p, \
             tc.tile_pool(name="recp", bufs=4) as recp, \
             tc.tile_pool(name="ysb", bufs=4) as ysb:
            if collective:
                y_part = dram.tile([S, E], f32, tag="ypart")
                rs_out = dram.tile([S // G, E], f32, tag="rsout")

            def new_ptt(h):
                return ptp.tile([128, KB * 512], C, tag="ptt", name=f"ptt{h}")

            # --- q-chunk 0 phase: K/Q projections interleaved with the
            # first head's score groups so ScalarE starts ASAP.
            ptts = {}
            emit_qk_proj(pall, wkT_sb, KT_sb, bk_sb, 0, 0)
            emit_qk_proj(pall, wqT_sb, QT_sb, bq_sb, 0, 0)
            ptts[0] = new_ptt(0)
            emit_scores_exp(pall, 0, 0, ptts[0], range(0, 2))
            for kc in range(1, 4):
                emit_qk_proj(pall, wkT_sb, KT_sb, bk_sb, 0, kc)
                emit_scores_exp(pall, 0, 0, ptts[0], range(2 * kc, 2 * kc + 2))
            ptts[1] = new_ptt(1)
            emit_scores_exp(pall, 1, 0, ptts[1], range(8))
            for kc in range(4):
                emit_qk_proj(pall, wkT_sb, KT_sb, bk_sb, 1, kc)
            emit_qk_proj(pall, wqT_sb, QT_sb, bq_sb, 1, 0)
            ptts[2] = new_ptt(2)
            emit_scores_exp(pall, 2, 0, ptts[2], range(8))
            for j in range(2):
                emit_v_proj_pair(pall, j)
            ptts[3] = new_ptt(3)
            emit_scores_exp(pall, 3, 0, ptts[3], range(8))
            for j in range(2, 4):
                emit_v_proj_pair(pall, j)
            emit_qk_proj(pall, wqT_sb, QT_sb, bq_sb, 0, 1)
            emit_qk_proj(pall, wqT_sb, QT_sb, bq_sb, 1, 1)

            # steady-state iterations: exps of qc overlap AV of qc-1,
            # transposes trail by a half-iteration, outproj by one more.
            O2s = {}
            ptts2 = None
            for qc in range(1, QC):
                pqc = qc - 1
                if qc >= 2:
                    emit_av(pall, recp, O2s[qc - 2], 3, ptts2[3])
                    emit_transposes(pall, O2s[qc - 2], qc - 2, 1)
                new_ptts = {}
                O2s[pqc] = [o2p.tile([128, GH * D], C, tag=f"o2_{qs}", name="o2")
                            for qs in range(4)]
                new_ptts[0] = new_ptt(0)
                emit_scores_exp(pall, 0, qc, new_ptts[0], range(8))
                if qc == 1:
                    for j in range(4, 8):
                        emit_v_proj_pair(pall, j)
                emit_av(pall, recp, O2s[pqc], 0, ptts[0])
                if qc >= 2:
                    for sb in range((qc - 2) * 4, (qc - 2) * 4 + 4):
                        emit_outproj_sb(pall, ysb, sb)
                new_ptts[1] = new_ptt(1)
                emit_scores_exp(pall, 1, qc, new_ptts[1], range(8))
                emit_av(pall, recp, O2s[pqc], 1, ptts[1])
                emit_transposes(pall, O2s[pqc], pqc, 0)
                new_ptts[2] = new_ptt(2)
                emit_scores_exp(pall, 2, qc, new_ptts[2], range(8))
                emit_av(pall, recp, O2s[pqc], 2, ptts[2])
                if qc < QC - 1:
                    emit_qk_proj(pall, wqT_sb, QT_sb, bq_sb, 0, qc + 1)
                    emit_qk_proj(pall, wqT_sb, QT_sb, bq_sb, 1, qc + 1)
                new_ptts[3] = new_ptt(3)
                emit_scores_exp(pall, 3, qc, new_ptts[3], range(8))
                ptts2 = ptts
                ptts = new_ptts

            # tail: finish qc2's head 3, then all of qc3
            pqc = QC - 1
            emit_av(pall, recp, O2s[QC - 2], 3, ptts2[3])
            emit_transposes(pall, O2s[QC - 2], QC - 2, 1)
            O2s[pqc] = [o2p.tile([128, GH * D], C, tag=f"o2_{qs}", name="o2")
                        for qs in range(4)]
            emit_av(pall, recp, O2s[pqc], 0, ptts[0])
            for sb in range((QC - 2) * 4, (QC - 2) * 4 + 4):
                emit_outproj_sb(pall, ysb, sb)
            emit_av(pall, recp, O2s[pqc], 1, ptts[1])
            emit_transposes(pall, O2s[pqc], pqc, 0)
            emit_av(pall, recp, O2s[pqc], 2, ptts[2])
            emit_av(pall, recp, O2s[pqc], 3, ptts[3])
            emit_transposes(pall, O2s[pqc], pqc, 1)
            for sb in range(pqc * 4, pqc * 4 + 4):
                emit_outproj_sb(pall, ysb, sb)

            if collective and do_coll:
                nc.gpsimd.collective_compute(
                    "ReduceScatter",
                    mybir.AluOpType.add,
                    replica_groups=[[0, 1, 2, 3], [4, 5, 6, 7]],
                    ins=[y_part.opt()],
                    outs=[rs_out.opt()],
                )
                nc.sync.dma_start(out=yout[:], in_=rs_out[:])

    with tile.TileContext(nc) as tc:
        with tc.tile_pool(name="res", bufs=1) as res:
            for _rep in range(reps):
                emit_body(nc, tc, res, do_coll=(_rep == reps - 1))
    nc.finalize()
    return nc


def _np_dtype(mode):
    if mode == "bf16":
        import ml_dtypes
        return ml_dtypes.bfloat16
    return np.float32


def _in_maps(query, Wq, bq, Wk, bk, Wv, bv, Wo, bo, mode):
    ndt = _np_dtype(mode)
    maps = []
    for c in range(NC):
        b, g = c // G, c % G
        gr = slice(g * GF, (g + 1) * GF)
        maps.append({
            "xT": np.ascontiguousarray(query[b].T).astype(ndt),
            "wqT": np.ascontiguousarray(Wq[gr, :].T).astype(ndt),
            "wkT": np.ascontiguousarray(Wk[gr, :].T).astype(ndt),
            "wvT": np.ascontiguousarray(Wv[gr, :].T).astype(ndt),
            "woT": np.ascontiguousarray(Wo[:, gr].T).astype(ndt),
            "bq_c": np.ascontiguousarray(
                np.asarray(bq[gr], np.float32).reshape(2, 128).T),
            "bk_c": np.ascontiguousarray(
                np.asarray(bk[gr], np.float32).reshape(2, 128).T),
            "bv_r": np.asarray(bv[gr]).reshape(1, GF).astype(ndt),
            "ones512": np.ones((1, 512), ndt),
            "ident": np.eye(128, dtype=np.float32).astype(ndt),
        })
    return maps


def kernel(query, Wq, bq, Wk, bk, Wv, bv, Wo, bo,
           mode="bf16", collective=True, trace=False):
    from concourse.bass_utils import run_bass_kernel_spmd

    key = (mode, collective, 1)
    if key not in _CACHE:
        _CACHE[key] = _build(mode, collective)
    nc = _CACHE[key]

    maps = _in_maps(query, Wq, bq, Wk, bk, Wv, bv, Wo, bo, mode)
    res = run_bass_kernel_spmd(nc, maps, list(range(NC)), trace=trace)

    out = np.empty((B, S, E), np.float32)
    if collective:
        for c in range(NC):
            b, g = c // G, c % G
            out[b, g * (S // G):(g + 1) * (S // G), :] = res.results[c]["yout"]
    else:
        for b in range(B):
            out[b] = sum(res.results[b * G + g]["yout"] for g in range(G))
    out += np.asarray(bo, np.float32)
    if trace:
        kernel.last_results = res
    return out


# revision 8
# speedup vs baseline: 1.2187x; 1.0094x over previous
"""Multi-head attention (B=2, S=2048, E=1024, H=16, D=64) on 8 trn2 cores.

Sharding: core c = (b, g) with b = c // 4 (batch), g = c % 4 (head group of
4 heads = 256 features). Each core computes Q/K/V projections for its head
group, full attention for its 4 heads, and a partial output projection
(columns of its group); a ReduceScatter over the 4 cores of each batch sums
the partials and leaves each core with a [512, 1024] slice of the final
output. The host concatenates the slices and adds bo.

Device-side layouts (host pre-transposes/casts):
  xT  [1024, 2048]  query[b].T                 (compute dtype)
  wqT/wkT/wvT [1024, 256]  W[g*256:(g+1)*256, :].T
  woT [256, 1024]          Wo[:, g*256:(g+1)*256].T
  bq_c/bk_c [128, 2]       bias columns (fp32, added in the PSUM->SBUF copy)
  bv_b [128, 256]          bias row pre-broadcast over partitions
  ident [128, 128]         identity for PE transposes

On-chip dataflow per core (all contractions on the partition dim):
  Q^T,K^T [f,s] = (W^T chunk).T @ x^T + bias   (bias via DVE tensor_scalar)
  V [k,f] = (x^T chunk).T @ W^T + bias         (bias via DVE tensor_tensor)
  S^T [k,q] = (K^T chunk).T @ Q^T   (K = d = 64)
  P^T = exp(S^T / 8)  via ScalarE, PSUM -> SBUF, cast to compute dtype
  O  [q,d+1] = (P^T chunk).T @ V'   with V' = [V | 1] (col d = denom)
    -- flipped AV: stationary = P^T chunk, so the matmul's free dim is
       d+1 = 65 instead of 512, quartering tensor-engine time there.
  O <- O * (1/denom)  (DVE per-partition scalar multiply), then
  O^T via PE transpose (identity), staged back to SBUF
  Y [s,f] = (O^T chunk).T @ Wo^T  -> ReduceScatter(+) over the 4-core group

Scheduling: engine queues are in-order, so emission order is everything.
x is DMAed in four column groups (the first K/Q tiles need only the first
quarter), the first head's score groups interleave with the K projection so
ScalarE starts ~11us in, and every iteration interleaves the previous
q-chunk's AV (split per 128-query subblock) between score groups.
"""

import numpy as np

B, S, E, H, D = 2, 2048, 1024, 16, 64
G = 4            # head groups (tensor-parallel)
GH = H // G      # heads per group = 4
GF = GH * D      # features per group = 256
NC = 8
SCALE = 1.0 / np.sqrt(D)

_CACHE = {}


def _build(mode: str, collective: bool, reps: int = 1):
    import concourse.bass as bass
    import concourse.mybir as mybir
    import concourse.tile as tile
    from concourse import bacc

    dt = mybir.dt
    C = {"bf16": dt.bfloat16, "f32r": dt.float32r, "fp32": dt.float32}[mode]
    f32 = dt.float32

    nc = bacc.Bacc()

    xT = nc.dram_tensor("xT", [E, S], C, kind="ExternalInput")
    wqT = nc.dram_tensor("wqT", [E, GF], C, kind="ExternalInput")
    wkT = nc.dram_tensor("wkT", [E, GF], C, kind="ExternalInput")
    wvT = nc.dram_tensor("wvT", [E, GF], C, kind="ExternalInput")
    woT = nc.dram_tensor("woT", [GF, E], C, kind="ExternalInput")
    bq_c = nc.dram_tensor("bq_c", [128, 2], f32, kind="ExternalInput")
    bk_c = nc.dram_tensor("bk_c", [128, 2], f32, kind="ExternalInput")
    bv_b = nc.dram_tensor("bv_b", [128, GF], C, kind="ExternalInput")
    ident = nc.dram_tensor("ident", [128, 128], C, kind="ExternalInput")
    if collective:
        yout = nc.dram_tensor("yout", [S // G, E], f32, kind="ExternalOutput")
    else:
        yout = nc.dram_tensor("yout", [S, E], f32, kind="ExternalOutput")

    EC = E // 128    # 8 e-chunks
    QC = S // 512    # 4 q-chunks
    KB = S // 128    # 16 k-blocks
    VW = GH * (D + 1)  # 260: V' row width (per head: 64 data + 1 ones col)

    def emit_body(nc, tc, res, do_coll):
        # x as one resident tile, e-chunk major; column-group DMAs fill it
        xAll = res.tile([128, EC * S], C, tag="xAll", name="xAll")

        def xs(ec, c0, c1):
            return xAll[:, ec * S + c0:ec * S + c1]

        wqT_sb = res.tile([128, EC * GF], C, tag="wqT")
        wkT_sb = res.tile([128, EC * GF], C, tag="wkT")
        wvT_sb = res.tile([128, EC * GF], C, tag="wvT")
        woT_sb = res.tile([128, 2 * E], C, tag="woT")
        QT_sb = [[res.tile([128, 512], C, tag=f"QT{fb}_{qc}", name=f"QT{fb}_{qc}")
                  for qc in range(QC)] for fb in range(2)]
        KT_sb = [[res.tile([128, 512], C, tag=f"KT{fb}_{qc}", name=f"KT{fb}_{qc}")
                  for qc in range(QC)] for fb in range(2)]
        V_sb = [res.tile([128, VW], C, tag=f"V{kb}", name=f"V{kb}")
                for kb in range(KB)]
        # O^T per qc: [128, 2*512]: free = hb*512 + q  (hb = head-pair block)
        OT2_sb = [res.tile([128, 2 * 512], C, tag=f"OT{qc}", name=f"OT{qc}")
                  for qc in range(QC)]
        bq_sb = res.tile([128, 2], f32, tag="bq")
        bk_sb = res.tile([128, 2], f32, tag="bk")
        bv_sb = res.tile([128, GF], C, tag="bv")
        id_sb = res.tile([128, 128], C, tag="ident")

        # input DMAs, ordered for the critical path: wk, x cols 0:512, wq,
        # qk biases, remaining x columns, then v/o weights and constants.
        def dma_w(dst, src, nech):
            nc.sync.dma_start(
                out=dst[:].rearrange("p (g f) -> p g f", g=nech),
                in_=src[:].rearrange("(g p) f -> p g f", p=128))

        def dma_xcols(qp):
            nc.sync.dma_start(
                out=xAll[:].rearrange("p (g s) -> p g s", g=EC)
                    [:, :, qp * 512:(qp + 1) * 512],
                in_=xT[:].rearrange("(g p) s -> p g s", p=128)
                    [:, :, qp * 512:(qp + 1) * 512])

        dma_w(wkT_sb, wkT, EC)
        dma_xcols(0)
        dma_w(wqT_sb, wqT, EC)
        nc.sync.dma_start(out=bk_sb[:], in_=bk_c[:])
        nc.sync.dma_start(out=bq_sb[:], in_=bq_c[:])
        for qp in range(1, 4):
            dma_xcols(qp)
        nc.sync.dma_start(out=bv_sb[:], in_=bv_b[:])
        nc.sync.dma_start(out=id_sb[:], in_=ident[:])
        dma_w(wvT_sb, wvT, EC)
        dma_w(woT_sb, woT, 2)

        # ones columns of V'
        for kb in range(KB):
            nc.gpsimd.memset(
                V_sb[kb][:].rearrange("p (h x) -> p h x", x=D + 1)[:, :, D:D + 1],
                1.0)

        # ---- emit helpers ----
        def emit_qk_proj(pp, w_sb, dst, b_sb, fb, qc):
            # Q^T / K^T group in [f, s] layout: stationary = W^T chunk
            pq = pp.tile([128, 512], f32, tag="pq", name="pq", bufs=2)
            for ec in range(EC):
                nc.tensor.matmul(
                    pq[:],
                    lhsT=w_sb[:, ec * GF + fb * 128:ec * GF + fb * 128 + 128],
                    rhs=xs(ec, qc * 512, qc * 512 + 512),
                    start=(ec == 0), stop=(ec == EC - 1))
            # bias folded into the PSUM->SBUF copy as a per-partition scalar
            nc.vector.tensor_scalar_add(
                out=dst[fb][qc][:], in0=pq[:], scalar1=b_sb[:, fb:fb + 1])

        def emit_v_proj_pair(pp, j):
            # V group in natural [k, f] layout for k-blocks 2j, 2j+1
            pv = pp.tile([128, 512], f32, tag="pav", name="pv", bufs=2)
            for t in range(2):
                kb = 2 * j + t
                for ec in range(EC):
                    nc.tensor.matmul(
                        pv[:, t * GF:(t + 1) * GF],
                        lhsT=xs(ec, kb * 128, kb * 128 + 128),
                        rhs=wvT_sb[:, ec * GF:(ec + 1) * GF],
                        start=(ec == 0), stop=(ec == EC - 1))
            for t in range(2):
                kb = 2 * j + t
                # bias via host-broadcast row, fused into the PSUM->SBUF copy
                nc.vector.tensor_tensor(
                    out=V_sb[kb][:].rearrange(
                        "p (h x) -> p h x", x=D + 1)[:, :, 0:D],
                    in0=pv[:, t * GF:(t + 1) * GF].rearrange(
                        "p (h d) -> p h d", d=D),
                    in1=bv_sb[:].rearrange("p (h d) -> p h d", d=D),
                    op=mybir.AluOpType.add)

        def emit_score_group(ps, h, qc, ptt, gi):
            hb, hr = h // 2, (h % 2) * D
            kb0 = 2 * gi
            pst = ps.tile([128, 2 * 512], f32, tag="pst", name="pst", bufs=2)
            for kj in range(2):
                kb = kb0 + kj
                nc.tensor.matmul(
                    pst[:, kj * 512:(kj + 1) * 512],
                    lhsT=KT_sb[hb][kb // 4][hr:hr + D,
                                            (kb % 4) * 128:(kb % 4) * 128 + 128],
                    rhs=QT_sb[hb][qc][hr:hr + D, :],
                    start=True, stop=True)
            nc.scalar.activation(
                ptt[:, kb0 * 512:(kb0 + 2) * 512],
                pst[:],
                mybir.ActivationFunctionType.Exp, scale=SCALE)

        def emit_av_qs(pav, recp, O2, h, ptt, qs):
            # Flipped AV for one 128-query subblock: stationary = P^T chunk
            # [128k, 128q], moving = V' [128k, 65]; accumulate over k-blocks.
            for kb in range(KB):
                nc.tensor.matmul(
                    pav[:, qs * (D + 1):(qs + 1) * (D + 1)],
                    lhsT=ptt[:, kb * 512 + qs * 128:kb * 512 + qs * 128 + 128],
                    rhs=V_sb[kb][:, h * (D + 1):(h + 1) * (D + 1)],
                    start=(kb == 0), stop=(kb == KB - 1))
            rec = recp.tile([128, 1], f32, tag="rec", name="rec")
            nc.vector.reciprocal(
                rec[:], pav[:, qs * (D + 1) + D:qs * (D + 1) + D + 1])
            nc.vector.tensor_scalar_mul(
                out=O2[qs][:, h * D:(h + 1) * D],
                in0=pav[:, qs * (D + 1):qs * (D + 1) + D],
                scalar1=rec[:])

        def emit_transposes(pp, O2, qc, hb):
            # O [q, f-pair] -> O^T [f-pair, q] for head pair hb, all 4 qs,
            # into a short-lived psum tile, then one DVE copy to OT2.
            pT = pp.tile([128, 512], C, tag="pq", name="pT")
            for qs in range(4):
                nc.tensor.transpose(
                    out=pT[:, qs * 128:qs * 128 + 128],
                    in_=O2[qs][:, hb * 128:hb * 128 + 128],
                    identity=id_sb[:])
            nc.vector.tensor_copy(
                OT2_sb[qc][:, hb * 512:(hb + 1) * 512], pT[:])

        def emit_outproj_sb(po, ysb, sb):
            qc = sb // 4
            for fc in range(2):
                pyt = po.tile([128, 512], f32, tag=("pav" if fc == 0 else "pq"),
                              name="pyt")
                for ec in range(2):
                    nc.tensor.matmul(
                        pyt[:],
                        lhsT=OT2_sb[qc][:, ec * 512 + (sb % 4) * 128:
                                        ec * 512 + (sb % 4) * 128 + 128],
                        rhs=woT_sb[:, ec * E + fc * 512:ec * E + fc * 512 + 512],
                        start=(ec == 0), stop=(ec == 1))
                yt = ysb.tile([128, 512], f32, tag="yt", name="yt")
                nc.vector.tensor_copy(yt[:], pyt[:])
                dst = y_part if collective else yout
                nc.sync.dma_start(
                    out=dst[sb * 128:(sb + 1) * 128, fc * 512:(fc + 1) * 512],
                    in_=yt[:])

        # ---- emission (order = scheduler priority; engine queues are
        # in-order, so every dependency must appear before its consumer,
        # and slow-to-unblock work must not be emitted ahead of urgent
        # work on the same engine) ----
        # PSUM (8 banks): "pst" [128,1024] x2 = 4 banks (scores+exp),
        # "pq" [128,512] x2 = 2 banks (projections, pT staging, outproj),
        # "pav" [128,512] x2 = 2 banks (V-proj, AV accum, outproj).
        with tc.tile_pool(name="dram", bufs=1, space="DRAM") as dram, \
             tc.tile_pool(name="pall", bufs=2, space="PSUM") as pall, \
             tc.tile_pool(name="ptp", bufs=6) as ptp, \
             tc.tile_pool(name="o2p", bufs=2) as o2p, \
             tc.tile_pool(name="recp", bufs=4) as recp, \
             tc.tile_pool(name="ysb", bufs=4) as ysb:
            if collective:
                y_part = dram.tile([S, E], f32, tag="ypart")
                rs_out = dram.tile([S // G, E], f32, tag="rsout")

            def new_ptt(h):
                return ptp.tile([128, KB * 512], C, tag="ptt", name=f"ptt{h}")

            def exp_block(h, qc, ptt, extras):
                """Emit the 8 score groups + exp calls for (h, qc), with
                `extras` (list of thunks) interleaved between groups."""
                for gi in range(8):
                    emit_score_group(pall, h, qc, ptt, gi)
                    if gi >= 1 and extras:
                        extras.pop(0)()
                while extras:
                    extras.pop(0)()

            # --- q-chunk 0 phase: K/Q projections interleaved with the
            # first heads' score groups so ScalarE starts ASAP.
            ptts = {}
            emit_qk_proj(pall, wkT_sb, KT_sb, bk_sb, 0, 0)
            emit_qk_proj(pall, wqT_sb, QT_sb, bq_sb, 0, 0)
            ptts[0] = new_ptt(0)
            exp_block(0, 0, ptts[0], [
                lambda: emit_qk_proj(pall, wkT_sb, KT_sb, bk_sb, 0, 1),
                lambda: emit_qk_proj(pall, wkT_sb, KT_sb, bk_sb, 0, 2),
                lambda: emit_qk_proj(pall, wkT_sb, KT_sb, bk_sb, 0, 3)])
            ptts[1] = new_ptt(1)
            exp_block(1, 0, ptts[1], [
                lambda: emit_qk_proj(pall, wkT_sb, KT_sb, bk_sb, 1, 0),
                lambda: emit_qk_proj(pall, wkT_sb, KT_sb, bk_sb, 1, 1),
                lambda: (emit_qk_proj(pall, wkT_sb, KT_sb, bk_sb, 1, 2),
                         emit_qk_proj(pall, wkT_sb, KT_sb, bk_sb, 1, 3),
                         emit_qk_proj(pall, wqT_sb, QT_sb, bq_sb, 1, 0))])
            ptts[2] = new_ptt(2)
            exp_block(2, 0, ptts[2], [
                lambda: emit_v_proj_pair(pall, 0),
                lambda: emit_v_proj_pair(pall, 1)])
            ptts[3] = new_ptt(3)
            exp_block(3, 0, ptts[3], [
                lambda: emit_v_proj_pair(pall, 2),
                lambda: emit_v_proj_pair(pall, 3),
                lambda: emit_qk_proj(pall, wqT_sb, QT_sb, bq_sb, 0, 1)])
            emit_qk_proj(pall, wqT_sb, QT_sb, bq_sb, 1, 1)

            # steady-state: exps of qc overlap AV of qc-1 (split per qs),
            # transposes of qc-1 complete within the iteration, outproj of
            # qc-2 rides along; the tail only owes qc3's AV + outproj.
            def av_extras(O2, h, ptt):
                # pav is allocated lazily at the first AV chunk so no other
                # same-tag allocation can slip between tile() and first write
                holder = {}

                def mk(qs):
                    def go():
                        if "pav" not in holder:
                            holder["pav"] = pall.tile(
                                [128, 512], f32, tag="pav", name="pav", bufs=2)
                        emit_av_qs(holder["pav"], recp, O2, h, ptt, qs)
                    return go
                return [mk(qs) for qs in range(4)]

            for qc in range(1, QC):
                pqc = qc - 1
                O2 = [o2p.tile([128, GH * D], C, tag=f"o2_{qs}", name="o2")
                      for qs in range(4)]
                new_ptts = {}
                new_ptts[0] = new_ptt(0)
                ex = av_extras(O2, 0, ptts[0])
                if qc == 1:
                    ex = [lambda: emit_v_proj_pair(pall, 4),
                          lambda: emit_v_proj_pair(pall, 5),
                          lambda: emit_v_proj_pair(pall, 6),
                          lambda: emit_v_proj_pair(pall, 7)] + ex
                exp_block(0, qc, new_ptts[0], ex)
                if qc >= 2:
                    for sb in range((qc - 2) * 4, (qc - 2) * 4 + 4):
                        emit_outproj_sb(pall, ysb, sb)
                new_ptts[1] = new_ptt(1)
                exp_block(1, qc, new_ptts[1], av_extras(O2, 1, ptts[1]))
                emit_transposes(pall, O2, pqc, 0)
                new_ptts[2] = new_ptt(2)
                ex = av_extras(O2, 2, ptts[2])
                if qc < QC - 1:
                    ex.append(lambda: emit_qk_proj(
                        pall, wqT_sb, QT_sb, bq_sb, 0, qc + 1))
                    ex.append(lambda: emit_qk_proj(
                        pall, wqT_sb, QT_sb, bq_sb, 1, qc + 1))
                exp_block(2, qc, new_ptts[2], ex)
                new_ptts[3] = new_ptt(3)
                exp_block(3, qc, new_ptts[3], av_extras(O2, 3, ptts[3]))
                emit_transposes(pall, O2, pqc, 1)
                if qc == QC - 1:
                    for sb in range((qc - 1) * 4, (qc - 1) * 4 + 4):
                        emit_outproj_sb(pall, ysb, sb)
                ptts = new_ptts

            # tail: qc3's AV + transposes + outproj
            pqc = QC - 1
            O2 = [o2p.tile([128, GH * D], C, tag=f"o2_{qs}", name="o2")
                  for qs in range(4)]
            for h in range(GH):
                for thunk in av_extras(O2, h, ptts[h]):
                    thunk()
                if h == 1:
                    emit_transposes(pall, O2, pqc, 0)
            emit_transposes(pall, O2, pqc, 1)
            for sb in range(pqc * 4, pqc * 4 + 4):
                emit_outproj_sb(pall, ysb, sb)

            if collective and do_coll:
                nc.gpsimd.collective_compute(
                    "ReduceScatter",
                    mybir.AluOpType.add,
                    replica_groups=[[0, 1, 2, 3], [4, 5, 6, 7]],
                    ins=[y_part.opt()],
                    outs=[rs_out.opt()],
                )
                nc.sync.dma_start(out=yout[:], in_=rs_out[:])

    with tile.TileContext(nc) as tc:
        with tc.tile_pool(name="res", bufs=1) as res:
            for _rep in range(reps):
                emit_body(nc, tc, res, do_coll=(_rep == reps - 1))
    nc.finalize()
    return nc


def _np_dtype(mode):
    if mode == "bf16":
        import ml_dtypes
        return ml_dtypes.bfloat16
    return np.float32


def _in_maps(query, Wq, bq, Wk, bk, Wv, bv, Wo, bo, mode):
    ndt = _np_dtype(mode)
    maps = []
    for c in range(NC):
        b, g = c // G, c % G
        gr = slice(g * GF, (g + 1) * GF)
        maps.append({
            "xT": np.ascontiguousarray(query[b].T).astype(ndt),
            "wqT": np.ascontiguousarray(Wq[gr, :].T).astype(ndt),
            "wkT": np.ascontiguousarray(Wk[gr, :].T).astype(ndt),
            "wvT": np.ascontiguousarray(Wv[gr, :].T).astype(ndt),
            "woT": np.ascontiguousarray(Wo[:, gr].T).astype(ndt),
            "bq_c": np.ascontiguousarray(
                np.asarray(bq[gr], np.float32).reshape(2, 128).T),
            "bk_c": np.ascontiguousarray(
                np.asarray(bk[gr], np.float32).reshape(2, 128).T),
            "bv_b": np.ascontiguousarray(
                np.tile(np.asarray(bv[gr]).reshape(1, GF), (128, 1))
            ).astype(ndt),
            "ident": np.eye(128, dtype=np.float32).astype(ndt),
        })
    return maps


def kernel(query, Wq, bq, Wk, bk, Wv, bv, Wo, bo,
           mode="bf16", collective=True, trace=False):
    from concourse.bass_utils import run_bass_kernel_spmd

    key = (mode, collective, 1)
    if key not in _CACHE:
        _CACHE[key] = _build(mode, collective)
    nc = _CACHE[key]

    maps = _in_maps(query, Wq, bq, Wk, bk, Wv, bv, Wo, bo, mode)
    res = run_bass_kernel_spmd(nc, maps, list(range(NC)), trace=trace)

    out = np.empty((B, S, E), np.float32)
    if collective:
        for c in range(NC):
            b, g = c // G, c % G
            out[b, g * (S // G):(g + 1) * (S // G), :] = res.results[c]["yout"]
    else:
        for b in range(B):
            out[b] = sum(res.results[b * G + g]["yout"] for g in range(G))
    out += np.asarray(bo, np.float32)
    if trace:
        kernel.last_results = res
    return out


# revision 13
# speedup vs baseline: 1.2357x; 1.0139x over previous
"""Multi-head attention (B=2, S=2048, E=1024, H=16, D=64) on 8 trn2 cores.

Sharding: core c = (b, g) with b = c // 4 (batch), g = c % 4 (head group of
4 heads = 256 features). Each core computes Q/K/V projections for its head
group, full attention for its 4 heads, and a partial output projection
(columns of its group); a ReduceScatter over the 4 cores of each batch sums
the partials and leaves each core with a [512, 1024] slice of the final
output. The host concatenates the slices and adds bo.

Device-side layouts (host pre-transposes/casts):
  xT  [1024, 2048]  query[b].T                 (compute dtype)
  wqT/wkT/wvT [1024, 256]  W[g*256:(g+1)*256, :].T
  woT [256, 1024]          Wo[:, g*256:(g+1)*256].T
  bq_c/bk_c [128, 2]       bias columns (fp32, added in the PSUM->SBUF copy)
  bv_b [128, 256]          bias row pre-broadcast over partitions
  ident [128, 128]         identity for PE transposes

On-chip dataflow per core (all contractions on the partition dim):
  Q^T,K^T [f,s] = (W^T chunk).T @ x^T + bias   (bias via DVE tensor_scalar)
  V [k,f] = (x^T chunk).T @ W^T + bias         (bias via DVE tensor_tensor)
  S^T [k,q] = (K^T chunk).T @ Q^T   (K = d = 64)
  P^T = exp(S^T / 8)  via ScalarE, PSUM -> SBUF, cast to compute dtype
  O  [q,d+1] = (P^T chunk).T @ V'   with V' = [V | 1] (col d = denom)
    -- flipped AV: stationary = P^T chunk, so the matmul's free dim is
       d+1 = 65 instead of 512, quartering tensor-engine time there.
  O <- O * (1/denom)  (DVE per-partition scalar multiply), then
  O^T via PE transpose (identity), staged back to SBUF
  Y [s,f] = (O^T chunk).T @ Wo^T  -> ReduceScatter(+) over the 4-core group

Scheduling: engine queues are in-order, so emission order is everything.
x is DMAed in four column groups (the first K/Q tiles need only the first
quarter), the first head's score groups interleave with the K projection so
ScalarE starts ~11us in, and every iteration interleaves the previous
q-chunk's AV (split per 128-query subblock) between score groups.
"""

import numpy as np

B, S, E, H, D = 2, 2048, 1024, 16, 64
G = 4            # head groups (tensor-parallel)
GH = H // G      # heads per group = 4
GF = GH * D      # features per group = 256
NC = 8
SCALE = 1.0 / np.sqrt(D)

_CACHE = {}


def _build(mode: str, collective: bool, reps: int = 1):
    import concourse.bass as bass
    import concourse.mybir as mybir
    import concourse.tile as tile
    from concourse import bacc

    dt = mybir.dt
    C = {"bf16": dt.bfloat16, "f32r": dt.float32r, "fp32": dt.float32}[mode]
    f32 = dt.float32

    nc = bacc.Bacc()

    xT = nc.dram_tensor("xT", [E, S], C, kind="ExternalInput")
    wqT = nc.dram_tensor("wqT", [E, GF], C, kind="ExternalInput")
    wkT = nc.dram_tensor("wkT", [E, GF], C, kind="ExternalInput")
    wvT = nc.dram_tensor("wvT", [E, GF], C, kind="ExternalInput")
    woT = nc.dram_tensor("woT", [GF, E], C, kind="ExternalInput")
    bkq_c = nc.dram_tensor("bkq_c", [128, 4], f32, kind="ExternalInput")
    bv_b = nc.dram_tensor("bv_b", [128, GF], C, kind="ExternalInput")
    if collective:
        yout = nc.dram_tensor("yout", [S // G, E], f32, kind="ExternalOutput")
    else:
        yout = nc.dram_tensor("yout", [S, E], f32, kind="ExternalOutput")

    EC = E // 128    # 8 e-chunks
    QC = S // 512    # 4 q-chunks
    KB = S // 128    # 16 k-blocks
    VW = GH * (D + 1)  # 260: V' row width (per head: 64 data + 1 ones col)

    def emit_body(nc, tc, res, do_coll):
        # x as one resident tile, e-chunk major; column-group DMAs fill it
        xAll = res.tile([128, EC * S], C, tag="xAll", name="xAll")

        def xs(ec, c0, c1):
            return xAll[:, ec * S + c0:ec * S + c1]

        wqT_sb = res.tile([128, EC * GF], C, tag="wqT")
        wkT_sb = res.tile([128, EC * GF], C, tag="wkT")
        wvT_sb = res.tile([128, EC * GF], C, tag="wvT")
        woT_sb = res.tile([128, 2 * E], C, tag="woT")
        QT_sb = [[res.tile([128, 512], C, tag=f"QT{fb}_{qc}", name=f"QT{fb}_{qc}")
                  for qc in range(QC)] for fb in range(2)]
        KT_sb = [[res.tile([128, 512], C, tag=f"KT{fb}_{qc}", name=f"KT{fb}_{qc}")
                  for qc in range(QC)] for fb in range(2)]
        V_sb = [res.tile([128, VW], C, tag=f"V{kb}", name=f"V{kb}")
                for kb in range(KB)]
        # O^T per qc: [128, 2*512]: free = hb*512 + q  (hb = head-pair block)
        OT2_sb = [res.tile([128, 2 * 512], C, tag=f"OT{qc}", name=f"OT{qc}")
                  for qc in range(QC)]
        bkq_sb = res.tile([128, 4], f32, tag="bkq")
        bk_sb = bkq_sb[:, 0:2]
        bq_sb = bkq_sb[:, 2:4]
        bv_sb = res.tile([128, GF], C, tag="bv")

        # input DMAs, ordered for the critical path: wk, x cols 0:512, wq,
        # qk biases, remaining x columns, then v/o weights and constants.
        def dma_w(dst, src, nech):
            nc.sync.dma_start(
                out=dst[:].rearrange("p (g f) -> p g f", g=nech),
                in_=src[:].rearrange("(g p) f -> p g f", p=128))

        def dma_xcols(qp):
            nc.sync.dma_start(
                out=xAll[:].rearrange("p (g s) -> p g s", g=EC)
                    [:, :, qp * 512:(qp + 1) * 512],
                in_=xT[:].rearrange("(g p) s -> p g s", p=128)
                    [:, :, qp * 512:(qp + 1) * 512])

        dma_w(wkT_sb, wkT, EC)
        dma_xcols(0)
        dma_w(wqT_sb, wqT, EC)
        nc.sync.dma_start(out=bkq_sb[:], in_=bkq_c[:])
        for qp in range(1, 4):
            dma_xcols(qp)
        nc.sync.dma_start(out=bv_sb[:], in_=bv_b[:])
        dma_w(wvT_sb, wvT, EC)
        dma_w(woT_sb, woT, 2)

        # ones columns of V'
        for kb in range(KB):
            nc.gpsimd.memset(
                V_sb[kb][:].rearrange("p (h x) -> p h x", x=D + 1)[:, :, D:D + 1],
                1.0)

        # ---- emit helpers ----
        def emit_qk_proj(pp, w_sb, dst, b_sb, fb, qc):
            # Q^T / K^T group in [f, s] layout: stationary = W^T chunk
            pq = pp.tile([128, 512], f32, tag="pq", name="pq", bufs=2)
            for ec in range(EC):
                nc.tensor.matmul(
                    pq[:],
                    lhsT=w_sb[:, ec * GF + fb * 128:ec * GF + fb * 128 + 128],
                    rhs=xs(ec, qc * 512, qc * 512 + 512),
                    start=(ec == 0), stop=(ec == EC - 1))
            # bias folded into the PSUM->SBUF copy as a per-partition scalar
            nc.vector.tensor_scalar_add(
                out=dst[fb][qc][:], in0=pq[:], scalar1=b_sb[:, fb:fb + 1])

        def emit_v_proj_pair(pp, j):
            # V group in natural [k, f] layout for k-blocks 2j, 2j+1
            pv = pp.tile([128, 512], f32, tag="pav", name="pv", bufs=2)
            for t in range(2):
                kb = 2 * j + t
                for ec in range(EC):
                    nc.tensor.matmul(
                        pv[:, t * GF:(t + 1) * GF],
                        lhsT=xs(ec, kb * 128, kb * 128 + 128),
                        rhs=wvT_sb[:, ec * GF:(ec + 1) * GF],
                        start=(ec == 0), stop=(ec == EC - 1))
            for t in range(2):
                kb = 2 * j + t
                # bias via host-broadcast row, fused into the PSUM->SBUF copy
                nc.vector.tensor_tensor(
                    out=V_sb[kb][:].rearrange(
                        "p (h x) -> p h x", x=D + 1)[:, :, 0:D],
                    in0=pv[:, t * GF:(t + 1) * GF].rearrange(
                        "p (h d) -> p h d", d=D),
                    in1=bv_sb[:].rearrange("p (h d) -> p h d", d=D),
                    op=mybir.AluOpType.add)

        def emit_score_group(ps, h, qc, ptt, gi):
            hb, hr = h // 2, (h % 2) * D
            kb0 = 2 * gi
            pst = ps.tile([128, 2 * 512], f32, tag="pst", name="pst", bufs=2)
            for kj in range(2):
                kb = kb0 + kj
                nc.tensor.matmul(
                    pst[:, kj * 512:(kj + 1) * 512],
                    lhsT=KT_sb[hb][kb // 4][hr:hr + D,
                                            (kb % 4) * 128:(kb % 4) * 128 + 128],
                    rhs=QT_sb[hb][qc][hr:hr + D, :],
                    start=True, stop=True)
            nc.scalar.activation(
                ptt[:, kb0 * 512:(kb0 + 2) * 512],
                pst[:],
                mybir.ActivationFunctionType.Exp, scale=SCALE)

        def emit_av_qs(pav, recp, O2, h, ptt, qs):
            # Flipped AV for one 128-query subblock: stationary = P^T chunk
            # [128k, 128q], moving = V' [128k, 65]; accumulate over k-blocks.
            for kb in range(KB):
                nc.tensor.matmul(
                    pav[:, qs * (D + 1):(qs + 1) * (D + 1)],
                    lhsT=ptt[:, kb * 512 + qs * 128:kb * 512 + qs * 128 + 128],
                    rhs=V_sb[kb][:, h * (D + 1):(h + 1) * (D + 1)],
                    start=(kb == 0), stop=(kb == KB - 1))
            rec = recp.tile([128, 1], f32, tag="rec", name="rec")
            nc.vector.reciprocal(
                rec[:], pav[:, qs * (D + 1) + D:qs * (D + 1) + D + 1])
            nc.vector.tensor_scalar_mul(
                out=O2[qs][:, h * D:(h + 1) * D],
                in0=pav[:, qs * (D + 1):qs * (D + 1) + D],
                scalar1=rec[:])

        def emit_transposes(pp, O2, qc, hb):
            # O [q, f-pair] -> O^T [f-pair, q] for head pair hb, all 4 qs,
            # via the DMA crossbar transpose (idle DMA engines, no PE/DVE)
            for qs in range(4):
                nc.sync.dma_start_transpose(
                    out=OT2_sb[qc][:, hb * 512 + qs * 128:
                                   hb * 512 + qs * 128 + 128],
                    in_=O2[qs][:, hb * 128:hb * 128 + 128])

        def emit_outproj_sb(po, ysb, sb):
            qc = sb // 4
            for fc in range(2):
                pyt = po.tile([128, 512], f32, tag=("pav" if fc == 0 else "pq"),
                              name="pyt")
                for ec in range(2):
                    nc.tensor.matmul(
                        pyt[:],
                        lhsT=OT2_sb[qc][:, ec * 512 + (sb % 4) * 128:
                                        ec * 512 + (sb % 4) * 128 + 128],
                        rhs=woT_sb[:, ec * E + fc * 512:ec * E + fc * 512 + 512],
                        start=(ec == 0), stop=(ec == 1))
                yt = ysb.tile([128, 512], f32, tag="yt", name="yt")
                nc.vector.tensor_copy(yt[:], pyt[:])
                dst = y_part if collective else yout
                nc.sync.dma_start(
                    out=dst[sb * 128:(sb + 1) * 128, fc * 512:(fc + 1) * 512],
                    in_=yt[:])

        # ---- emission (order = scheduler priority; engine queues are
        # in-order, so every dependency must appear before its consumer,
        # and slow-to-unblock work must not be emitted ahead of urgent
        # work on the same engine) ----
        # PSUM (8 banks): "pst" [128,1024] x2 = 4 banks (scores+exp),
        # "pq" [128,512] x2 = 2 banks (projections, pT staging, outproj),
        # "pav" [128,512] x2 = 2 banks (V-proj, AV accum, outproj).
        with tc.tile_pool(name="dram", bufs=1, space="DRAM") as dram, \
             tc.tile_pool(name="pall", bufs=2, space="PSUM") as pall, \
             tc.tile_pool(name="ptp", bufs=6) as ptp, \
             tc.tile_pool(name="o2p", bufs=2) as o2p, \
             tc.tile_pool(name="recp", bufs=4) as recp, \
             tc.tile_pool(name="ysb", bufs=4) as ysb:
            if collective:
                y_part = dram.tile([S, E], f32, tag="ypart")
                rs_out = dram.tile([S // G, E], f32, tag="rsout")

            def new_ptt(h):
                return ptp.tile([128, KB * 512], C, tag="ptt", name=f"ptt{h}")

            def exp_block(h, qc, ptt, extras):
                """Emit the 8 score groups + exp calls for (h, qc), with
                `extras` (list of thunks) interleaved between groups."""
                for gi in range(8):
                    emit_score_group(pall, h, qc, ptt, gi)
                    if gi >= 1 and extras:
                        extras.pop(0)()
                while extras:
                    extras.pop(0)()

            # --- q-chunk 0 phase: K/Q projections interleaved with the
            # first heads' score groups so ScalarE starts ASAP.
            ptts = {}
            emit_qk_proj(pall, wkT_sb, KT_sb, bk_sb, 0, 0)
            emit_qk_proj(pall, wqT_sb, QT_sb, bq_sb, 0, 0)
            ptts[0] = new_ptt(0)
            exp_block(0, 0, ptts[0], [
                lambda: emit_qk_proj(pall, wkT_sb, KT_sb, bk_sb, 0, 1),
                lambda: emit_qk_proj(pall, wkT_sb, KT_sb, bk_sb, 0, 2),
                lambda: emit_qk_proj(pall, wkT_sb, KT_sb, bk_sb, 0, 3)])
            ptts[1] = new_ptt(1)
            exp_block(1, 0, ptts[1], [
                lambda: emit_qk_proj(pall, wkT_sb, KT_sb, bk_sb, 1, 0),
                lambda: emit_qk_proj(pall, wkT_sb, KT_sb, bk_sb, 1, 1),
                lambda: (emit_qk_proj(pall, wkT_sb, KT_sb, bk_sb, 1, 2),
                         emit_qk_proj(pall, wkT_sb, KT_sb, bk_sb, 1, 3),
                         emit_qk_proj(pall, wqT_sb, QT_sb, bq_sb, 1, 0))])
            ptts[2] = new_ptt(2)
            exp_block(2, 0, ptts[2], [
                lambda: emit_v_proj_pair(pall, 0),
                lambda: emit_v_proj_pair(pall, 1)])
            ptts[3] = new_ptt(3)
            exp_block(3, 0, ptts[3], [
                lambda: emit_v_proj_pair(pall, 2),
                lambda: emit_v_proj_pair(pall, 3),
                lambda: emit_qk_proj(pall, wqT_sb, QT_sb, bq_sb, 0, 1)])
            emit_qk_proj(pall, wqT_sb, QT_sb, bq_sb, 1, 1)

            # steady-state: exps of qc overlap AV of qc-1 (split per qs),
            # transposes of qc-1 complete within the iteration, outproj of
            # qc-2 rides along; the tail only owes qc3's AV + outproj.
            def av_extras(O2, h, ptt):
                # pav is allocated lazily at the first AV chunk so no other
                # same-tag allocation can slip between tile() and first write
                holder = {}

                def mk(qs):
                    def go():
                        if "pav" not in holder:
                            holder["pav"] = pall.tile(
                                [128, 512], f32, tag="pav", name="pav", bufs=2)
                        emit_av_qs(holder["pav"], recp, O2, h, ptt, qs)
                    return go
                return [mk(qs) for qs in range(4)]

            for qc in range(1, QC):
                pqc = qc - 1
                O2 = [o2p.tile([128, GH * D], C, tag=f"o2_{qs}", name="o2")
                      for qs in range(4)]
                new_ptts = {}
                new_ptts[0] = new_ptt(0)
                ex = av_extras(O2, 0, ptts[0])
                if qc == 1:
                    ex = [lambda: emit_v_proj_pair(pall, 4),
                          lambda: emit_v_proj_pair(pall, 5),
                          lambda: emit_v_proj_pair(pall, 6),
                          lambda: emit_v_proj_pair(pall, 7)] + ex
                exp_block(0, qc, new_ptts[0], ex)
                if qc >= 2:
                    for sb in range((qc - 2) * 4, (qc - 2) * 4 + 4):
                        emit_outproj_sb(pall, ysb, sb)
                new_ptts[1] = new_ptt(1)
                exp_block(1, qc, new_ptts[1], av_extras(O2, 1, ptts[1]))
                emit_transposes(pall, O2, pqc, 0)
                new_ptts[2] = new_ptt(2)
                ex = av_extras(O2, 2, ptts[2])
                if qc < QC - 1:
                    ex.append(lambda: emit_qk_proj(
                        pall, wqT_sb, QT_sb, bq_sb, 0, qc + 1))
                    ex.append(lambda: emit_qk_proj(
                        pall, wqT_sb, QT_sb, bq_sb, 1, qc + 1))
                exp_block(2, qc, new_ptts[2], ex)
                new_ptts[3] = new_ptt(3)
                exp_block(3, qc, new_ptts[3], av_extras(O2, 3, ptts[3]))
                emit_transposes(pall, O2, pqc, 1)
                if qc == QC - 1:
                    for sb in range((qc - 1) * 4, (qc - 1) * 4 + 4):
                        emit_outproj_sb(pall, ysb, sb)
                ptts = new_ptts

            # tail: qc3's AV + transposes + outproj
            pqc = QC - 1
            O2 = [o2p.tile([128, GH * D], C, tag=f"o2_{qs}", name="o2")
                  for qs in range(4)]
            for h in range(GH):
                for thunk in av_extras(O2, h, ptts[h]):
                    thunk()
                if h == 1:
                    emit_transposes(pall, O2, pqc, 0)
            emit_transposes(pall, O2, pqc, 1)
            for sb in range(pqc * 4, pqc * 4 + 4):
                emit_outproj_sb(pall, ysb, sb)

            if collective and do_coll:
                nc.gpsimd.collective_compute(
                    "ReduceScatter",
                    mybir.AluOpType.add,
                    replica_groups=[[0, 1, 2, 3], [4, 5, 6, 7]],
                    ins=[y_part.opt()],
                    outs=[rs_out.opt()],
                )
                nc.sync.dma_start(out=yout[:], in_=rs_out[:])

    with tile.TileContext(nc) as tc:
        with tc.tile_pool(name="res", bufs=1) as res:
            for _rep in range(reps):
                emit_body(nc, tc, res, do_coll=(_rep == reps - 1))
    nc.finalize()
    return nc


def _np_dtype(mode):
    if mode == "bf16":
        import ml_dtypes
        return ml_dtypes.bfloat16
    return np.float32


def _in_maps(query, Wq, bq, Wk, bk, Wv, bv, Wo, bo, mode):
    ndt = _np_dtype(mode)
    maps = []
    for c in range(NC):
        b, g = c // G, c % G
        gr = slice(g * GF, (g + 1) * GF)
        maps.append({
            "xT": np.ascontiguousarray(query[b].T).astype(ndt),
            "wqT": np.ascontiguousarray(Wq[gr, :].T).astype(ndt),
            "wkT": np.ascontiguousarray(Wk[gr, :].T).astype(ndt),
            "wvT": np.ascontiguousarray(Wv[gr, :].T).astype(ndt),
            "woT": np.ascontiguousarray(Wo[:, gr].T).astype(ndt),
            "bkq_c": np.ascontiguousarray(np.concatenate([
                np.asarray(bk[gr], np.float32).reshape(2, 128).T,
                np.asarray(bq[gr], np.float32).reshape(2, 128).T], axis=1)),
            "bv_b": np.ascontiguousarray(
                np.tile(np.asarray(bv[gr]).reshape(1, GF), (128, 1))
            ).astype(ndt),
        })
    return maps


def kernel(query, Wq, bq, Wk, bk, Wv, bv, Wo, bo,
           mode="bf16", collective=True, trace=False):
    from concourse.bass_utils import run_bass_kernel_spmd

    key = (mode, collective, 1)
    if key not in _CACHE:
        _CACHE[key] = _build(mode, collective)
    nc = _CACHE[key]

    maps = _in_maps(query, Wq, bq, Wk, bk, Wv, bv, Wo, bo, mode)
    res = run_bass_kernel_spmd(nc, maps, list(range(NC)), trace=trace)

    out = np.empty((B, S, E), np.float32)
    if collective:
        for c in range(NC):
            b, g = c // G, c % G
            out[b, g * (S // G):(g + 1) * (S // G), :] = res.results[c]["yout"]
    else:
        for b in range(B):
            out[b] = sum(res.results[b * G + g]["yout"] for g in range(G))
    out += np.asarray(bo, np.float32)
    if trace:
        kernel.last_results = res
    return out
